# revision 16
# baseline (speedup 1.0000x reference)
"""EnhancedGraphSAGE on 8 trn2 NeuronCores (Bass/Tile).

Sharding: 8 graphs per core (batch is sorted -> nodes graph-contiguous).
Each graph padded to G_slot slots (multiple of 128) with phantom nodes that
clone the graph's first node's features (no in-edges), so windows are
graph-pure with fully static shapes. h >= 0 always (relu encoder + relu
residual), so zeroing pad columns before pooling makes sum/max exact.
h is replicated across cores (fp16) via AllGather after the encoder and
after each SAGE layer.

Mean aggregation: per-core edges are grouped by (dst group of 4 windows,
src bank) into 128-edge chunks. dma_gather (int16 idx, 4 DRAM banks of the
fp16 replicated h, one SWDGE queue per bank) pulls h[src] rows into SBUF;
the PE accumulates aggT[f, slot-in-group] per 512-slot group as
gathered.T @ onehot where onehot[e, n] = (dlocal[e]==n) * invdeg[dst_e]
(fp16) is built on DVE with one fused tensor_scalar per chunk.
hn = agg@Wl + bl + h@Wr runs from aggT / resident hT (feature-major),
LN + relu + residual in node-major.
"""

import math
import os
from contextlib import ExitStack

import numpy as np

H = 128
HT = 64
NCLS = 8
L = 3
P = 128
NCORES = 8
GPC = 8  # graphs per core
GRPW = 4  # windows per psum group (512 dst slots)
GW = GRPW * P
SGG = 4  # groups per supergroup (one gather call per bank per supergroup)
MAX_BANK_ROWS = 32767
NBANKS = 4


# ----------------------------------------------------------------------------
# host-side schedule construction
# ----------------------------------------------------------------------------

def _build_schedule(x, edge_index, batch):
    N = x.shape[0]
    E = edge_index.shape[1]
    B = GPC * NCORES
    cnt = np.bincount(batch, minlength=B)
    assert cnt.min() > 0, "empty graph unsupported"
    gstart = np.zeros(B + 1, np.int64)
    np.cumsum(cnt, out=gstart[1:])
    G_slot = int(math.ceil(cnt.max() / P) * P)
    S = GPC * G_slot          # padded slots per core
    W = S // P                # windows per core
    WG = G_slot // P          # windows per graph
    assert S % GW == 0
    ngroups = S // GW
    bank_rows = int(math.ceil(NCORES * S / NBANKS))
    assert bank_rows <= MAX_BANK_ROWS

    g_of = batch.astype(np.int64)
    core_of_g = np.arange(B) // GPC
    slot_in_core_base = (np.arange(B) % GPC) * G_slot
    # per-core slot of real node n, and global replicated position
    slot = slot_in_core_base[g_of] + (np.arange(N) - gstart[g_of])
    p_rep = core_of_g[g_of] * S + slot

    src = edge_index[0].astype(np.int64)
    dst = edge_index[1].astype(np.int64)
    deg = np.bincount(dst, minlength=N).astype(np.float64)
    invdeg_node = 1.0 / np.maximum(deg, 1.0)

    e_core = core_of_g[g_of[dst]]
    e_slot = slot[dst]
    e_psrc = p_rep[src]
    e_inv = invdeg_node[dst]

    e_g = e_slot // GW        # dst group within core
    e_dl = e_slot % GW        # dst slot within group
    e_bank = e_psrc // bank_rows
    e_idx = e_psrc % bank_rows

    # per (core, group, bank) cell edge lists; chunk count = max over cores
    key = ((e_core * ngroups + e_g) * NBANKS + e_bank).astype(np.int64)
    korder = np.argsort(key, kind="stable")
    ks = key[korder]
    bounds = np.searchsorted(ks, np.arange(NCORES * ngroups * NBANKS + 1))

    def cell_list(c, g, b):
        k = (c * ngroups + g) * NBANKS + b
        return korder[bounds[k]:bounds[k + 1]]

    cell_chunks = np.zeros((ngroups, NBANKS), np.int64)
    for g in range(ngroups):
        for b in range(NBANKS):
            m = max(len(cell_list(c, g, b)) for c in range(NCORES))
            cell_chunks[g, b] = (m + P - 1) // P
    nchunks = int(cell_chunks.sum())

    # supergroups: SGG consecutive groups share one gather call per bank
    sgs = [list(range(s, min(s + SGG, ngroups))) for s in range(0, ngroups, SGG)]
    # chunk base of cell (g, b) within its (sg, b) call
    cell_base = {}
    call_nch = {}  # (si, b) -> chunks in call
    for si, sg in enumerate(sgs):
        for b in range(NBANKS):
            ofs = 0
            for g in sg:
                cell_base[(g, b)] = ofs
                ofs += int(cell_chunks[g, b])
            call_nch[(si, b)] = ofs

    # pack per-core idx / dlocal / invdegE in emission order
    # (each call's idx region 64B-aligned: 32 int16 cols)
    def _acols(nch):
        return -(-int(nch) * P // 16 // 32) * 32

    total_idx_cols = sum(_acols(call_nch[(si, b)])
                         for si in range(len(sgs)) for b in range(NBANKS))
    idx16 = np.zeros((NCORES, 128, total_idx_cols), np.int16)
    dlocal = np.full((NCORES, P, nchunks), 30000.0, np.float32)
    invdegE = np.zeros((NCORES, P, nchunks), np.float32)

    # global chunk offset of cell (g, b) in dlocal/invdegE, in emission order
    chunk_ofs = {}
    ofs = 0
    for si, sg in enumerate(sgs):
        for b in range(NBANKS):
            for g in sg:
                chunk_ofs[(g, b)] = ofs
                ofs += int(cell_chunks[g, b])
    assert ofs == nchunks

    for c in range(NCORES):
        colofs = 0
        for si, sg in enumerate(sgs):
            for b in range(NBANKS):
                nch_call = call_nch[(si, b)]
                if nch_call == 0:
                    continue
                vals = np.zeros(nch_call * P, np.int64)
                for g in sg:
                    nch = int(cell_chunks[g, b])
                    if nch == 0:
                        continue
                    lst = cell_list(c, g, b)
                    n = len(lst)
                    cb = cell_base[(g, b)]
                    vals[cb * P: cb * P + n] = e_idx[lst]
                    dlf = np.full(nch * P, 999.0, np.float32)
                    ivf = np.zeros(nch * P, np.float32)
                    dlf[:n] = e_dl[lst]
                    ivf[:n] = e_inv[lst]
                    co = chunk_ofs[(g, b)]
                    dlocal[c, :, co:co + nch] = dlf.reshape(nch, P).T
                    invdegE[c, :, co:co + nch] = ivf.reshape(nch, P).T
                ncols = nch_call * P // 16
                wrapped = vals.reshape(ncols, 16).T.astype(np.int16)
                for r in range(8):
                    idx16[c, r * 16:(r + 1) * 16, colofs:colofs + ncols] = wrapped
                colofs += _acols(nch_call)

    # first/last (bank, chunk) per group for PSUM start/stop
    grp_first = {}
    grp_last = {}
    for g in range(ngroups):
        nz = [(b, int(cell_chunks[g, b])) for b in range(NBANKS)
              if cell_chunks[g, b] > 0]
        if nz:
            grp_first[g] = (nz[0][0], 0)
            grp_last[g] = (nz[-1][0], nz[-1][1] - 1)

    return dict(
        N=N, E=E, B=B, cnt=cnt, gstart=gstart, G_slot=G_slot, S=S, W=W,
        WG=WG, bank_rows=bank_rows, slot=slot,
        cell_chunks=cell_chunks, nchunks=nchunks, sgs=sgs,
        cell_base=cell_base, call_nch=call_nch, chunk_ofs=chunk_ofs,
        idx16=idx16, dlocal=dlocal, invdegE=invdegE,
        total_idx_cols=total_idx_cols,
        grp_first=grp_first, grp_last=grp_last, ngroups=ngroups,
    )


def _host_inputs(sched, x, ts, weights):
    """Per-core input dicts (plus shared tensors replicated)."""
    S, G_slot = sched["S"], sched["G_slot"]
    cnt, gstart = sched["cnt"], sched["gstart"]
    slot = sched["slot"]

    xT = np.zeros((NCORES, 4, S), np.float32)
    g_all = np.repeat(np.arange(sched["B"]), cnt)
    for c in range(NCORES):
        sel = (g_all // GPC) == c
        xT[c, :, slot[sel]] = x[sel]
    # phantoms copy n0's features (keeps pad h finite; no in-edges)
    for g in range(sched["B"]):
        c = g // GPC
        base = (g % GPC) * G_slot
        nph = G_slot - cnt[g]
        if nph > 0:
            xT[c, :, base + cnt[g]: base + G_slot] = x[gstart[g]][:, None]

    invcnt = np.zeros((NCORES, GPC), np.float32)
    for g in range(sched["B"]):
        invcnt[g // GPC, g % GPC] = 1.0 / cnt[g]

    W = sched["W"]
    padmask = np.zeros((NCORES, P, W), np.float32)
    for g in range(sched["B"]):
        c = g // GPC
        base = (g % GPC) * G_slot
        real = np.arange(base, base + cnt[g])
        padmask[c, real % P, real // P] = 1.0

    iota512 = np.tile(np.arange(GW, dtype=np.float16), (P, 1))
    ident = np.eye(P, dtype=np.float32)

    per_core = []
    for c in range(NCORES):
        d = {
            "xT": np.ascontiguousarray(xT[c]),
            "gidx": np.ascontiguousarray(sched["idx16"][c]),
            "dlocal": np.ascontiguousarray(sched["dlocal"][c]),
            "invdegE": np.ascontiguousarray(sched["invdegE"][c]),
            "tsT": np.ascontiguousarray(ts[c * GPC:(c + 1) * GPC].T.astype(np.float32)),
            "invcnt": invcnt[c:c + 1],
            "padmask": np.ascontiguousarray(padmask[c]),
            "iota512": iota512,
            "ident": ident,
        }
        for k, v in weights.items():
            d[k] = v
        per_core.append(d)
    return per_core


# ----------------------------------------------------------------------------
# bass program
# ----------------------------------------------------------------------------

def _build_nc(sched):
    import concourse.bacc as bacc
    import concourse.bass as bass
    import concourse.mybir as mybir
    import concourse.tile as tile
    from concourse import library_config

    f32 = mybir.dt.float32
    f16 = mybir.dt.float16
    AF = mybir.ActivationFunctionType
    OP = mybir.AluOpType

    S, W, WG = sched["S"], sched["W"], sched["WG"]
    bank_rows = sched["bank_rows"]
    ngroups = sched["ngroups"]
    cell_chunks = sched["cell_chunks"]
    sgs = sched["sgs"]
    cell_base = sched["cell_base"]
    call_nch = sched["call_nch"]
    chunk_ofs = sched["chunk_ofs"]
    nchunks = sched["nchunks"]
    total_idx_cols = sched["total_idx_cols"]
    grp_first, grp_last = sched["grp_first"], sched["grp_last"]

    stage = os.environ.get("GNN_STAGE", "full")
    flags = set(stage.split("+"))
    nc = bacc.Bacc("TRN2", target_bir_lowering=False, num_swdge_queues=NBANKS)

    def din(name, shape, dtype=f32):
        return nc.dram_tensor(name, shape, dtype, kind="ExternalInput")

    xT_d = din("xT", [4, S])
    gidx_d = din("gidx", [128, total_idx_cols], mybir.dt.int16)
    dlocal_d = din("dlocal", [P, nchunks])
    invdegE_d = din("invdegE", [P, nchunks])
    tsT_d = din("tsT", [3, GPC])
    invcnt_d = din("invcnt", [1, GPC])
    padmask_d = din("padmask", [P, W])
    iota512_d = din("iota512", [P, GW], f16)
    ident_d = din("ident", [P, P])
    encW_d = din("enc_W", [4, H])
    encb_d = din("enc_b", [H])
    Wl_d = din("sage_Wl", [L * H, H], f16)
    bl_d = din("sage_bl", [L, H])
    Wr_d = din("sage_Wr", [L * H, H])
    lng_d = din("ln_g", [L, H])
    lnb_d = din("ln_b", [L, H])
    tsW1_d = din("ts_W1", [3, HT])
    tsb1_d = din("ts_b1", [HT])
    tslng_d = din("ts_lng", [HT])
    tslnb_d = din("ts_lnb", [HT])
    tsW2_d = din("ts_W2", [HT, HT])
    tsb2_d = din("ts_b2", [HT])
    clng_d = din("cls_lng", [2 * H + HT])
    clnb_d = din("cls_lnb", [2 * H + HT])
    cW1_d = din("cls_W1", [2 * H + HT, H])
    cb1_d = din("cls_b1", [H])
    cW2_d = din("cls_W2", [H, NCLS])
    cb2_d = din("cls_b2", [NCLS])
    out_d = nc.dram_tensor("out", [GPC, NCLS], f32, kind="ExternalOutput")

    h_shard = [nc.dram_tensor(f"h_shard{l}", [S, H], f16) for l in range(L)]
    h_rep = [nc.dram_tensor(f"h_rep{l}", [NCORES * S, H], f16,
                            addr_space="Shared") for l in range(L)]
    # one-hot cache: built on DVE in layer 0, streamed back in layers 1+
    oh_dram = nc.dram_tensor("oh_cache", [nchunks * P, GW], f16)

    def bcast_row(dram_ap, npart, width):
        # AP reading a [width] or [1,width] dram row replicated across npart partitions
        return bass.AP(tensor=dram_ap.tensor, offset=dram_ap.offset,
                       ap=[[0, npart]] + dram_ap.ap[-1:])

    with tile.TileContext(nc) as tc, ExitStack() as ctx:
        res = ctx.enter_context(tc.tile_pool(name="res", bufs=1))
        gath = ctx.enter_context(tc.tile_pool(name="gath", bufs=6))
        oh = ctx.enter_context(tc.tile_pool(name="oh", bufs=4))
        stg = ctx.enter_context(tc.tile_pool(name="stg", bufs=4))
        sml = ctx.enter_context(tc.tile_pool(name="sml", bufs=2))
        ps_agg = ctx.enter_context(tc.tile_pool(name="ps_agg", bufs=2, space="PSUM"))
        ps_hn = ctx.enter_context(tc.tile_pool(name="ps_hn", bufs=2, space="PSUM"))
        ps_t = ctx.enter_context(tc.tile_pool(name="ps_t", bufs=2, space="PSUM"))

        nc.gpsimd.load_library(library_config.mlp)

        # ---- residents ----
        hT = res.tile([P, S], f32)                      # feature-major h shard
        gidx_s = res.tile([128, total_idx_cols], mybir.dt.int16)
        dl_s = res.tile([P, nchunks], f32)
        iv_s = res.tile([P, nchunks], f32)
        iota_s = res.tile([P, GW], f16)
        ident_s = res.tile([P, P], f32)
        encW_s = res.tile([4, H], f32)
        encb_c = res.tile([P, 1], f32)
        eps_c = res.tile([P, 1], f32)
        padmask_s = res.tile([P, W], f32)
        nc.sync.dma_start(padmask_s[:], padmask_d[:])
        nc.sync.dma_start(gidx_s[:], gidx_d[:])
        nc.sync.dma_start(dl_s[:], dlocal_d[:])
        nc.sync.dma_start(iv_s[:], invdegE_d[:])
        nc.sync.dma_start(iota_s[:], iota512_d[:])
        nc.sync.dma_start(ident_s[:], ident_d[:])
        nc.sync.dma_start(encW_s[:], encW_d[:])
        nc.sync.dma_start(encb_c[:], encb_d.ap().rearrange("h -> h ()"))
        nc.vector.memset(eps_c[:], 1e-5)

        REPS = int(os.environ.get("GNN_REPS", "1"))

        def _pipeline():
            # ---- encoder: hT = relu(enc_W.T @ xT + b) ----
            for w in range(W):
                sl = slice(w * P, (w + 1) * P)
                xw = stg.tile([4, P], f32, tag="xw")
                nc.sync.dma_start(xw[:], xT_d[:, sl])
                ps = ps_hn.tile([P, P], f32, tag="ph")
                nc.tensor.matmul(ps[:], lhsT=encW_s[:], rhs=xw[:],
                                 start=True, stop=True)
                nc.scalar.activation(hT[:, sl], ps[:], AF.Relu, bias=encb_c[:])
                pt = ps_t.tile([P, P], f32, tag="pt")
                nc.tensor.transpose(pt[:], hT[:, sl], ident_s[:])
                st = stg.tile([P, P], f16, tag="st16")
                nc.scalar.activation(st[:], pt[:], AF.Copy)
                nc.sync.dma_start(h_shard[0][sl, :], st[:])
            if not flags & {"noag", "nolayers"}:
                nc.gpsimd.collective_compute(
                    "AllGather", OP.bypass, ins=[h_shard[0].ap()],
                    outs=[h_rep[0].ap()], replica_groups=[list(range(NCORES))])

            # ---- SAGE layers ----
            for l in range(L if "nolayers" not in flags else 0):
                Wl_s = sml.tile([H, H], f16, tag="wl")
                Wr_s = sml.tile([H, H], f32, tag="wr")
                blb = sml.tile([P, H], f32, tag="blb")
                gb = sml.tile([P, H], f32, tag="gb")
                bb = sml.tile([P, H], f32, tag="bb")
                nc.sync.dma_start(Wl_s[:], Wl_d[l * H:(l + 1) * H, :])
                nc.sync.dma_start(Wr_s[:], Wr_d[l * H:(l + 1) * H, :])
                nc.sync.dma_start(blb[:], bcast_row(bl_d[l, :], P, H))
                nc.sync.dma_start(gb[:], bcast_row(lng_d[l, :], P, H))
                nc.sync.dma_start(bb[:], bcast_row(lnb_d[l, :], P, H))

                LREPS = int(os.environ.get("GNN_LREPS", "1"))
                for _lr in range(LREPS):
                  colofs = 0
                  for si, sg in enumerate(sgs):
                    gtiles = {}
                    for b in range(NBANKS):
                        nch_call = call_nch[(si, b)]
                        if nch_call == 0:
                            continue
                        ncols = nch_call * P // 16
                        acols = -(-ncols // 32) * 32
                        if "nogather" not in flags:
                            gt = gath.tile([P, nch_call, P], f16, tag="gath")
                            nc.gpsimd.dma_gather(
                                gt[:], h_rep[l][b * bank_rows:(b + 1) * bank_rows, :],
                                gidx_s[:, colofs:colofs + ncols],
                                nch_call * P, nch_call * P, H,
                                single_packet=(nch_call * P <= 1024),
                                queue_num=b)
                            gtiles[b] = gt
                        colofs += acols
                    for g in sg:
                      # chunk matmuls: bank-major, accumulating aggT per group
                      psw = None
                      if g in grp_first and not flags & {"nogather", "gatheronly"}:
                          psw = ps_agg.tile([P, GW], f32, tag="aggw")
                      for b in range(NBANKS):
                          if b not in gtiles or "gatheronly" in flags:
                              continue
                          assert "nogather" not in flags
                          nch = int(cell_chunks[g, b])
                          cb = cell_base[(g, b)]
                          co = chunk_ofs[(g, b)]
                          for c in range(nch):
                              ci = co + c
                              ohc = oh.tile([P, GW], f16, tag="oh")
                              if l == 0 and _lr == 0:
                                  nc.vector.tensor_scalar(
                                      ohc[:], iota_s[:], dl_s[:, ci:ci + 1],
                                      iv_s[:, ci:ci + 1], OP.is_equal, OP.mult)
                                  nc.sync.dma_start(
                                      oh_dram[ci * P:(ci + 1) * P, :], ohc[:])
                              else:
                                  nc.sync.dma_start(
                                      ohc[:], oh_dram[ci * P:(ci + 1) * P, :])
                              nc.tensor.matmul(
                                  psw[:], lhsT=gtiles[b][:, cb + c, :],
                                  rhs=ohc[:],
                                  start=(grp_first[g] == (b, c)),
                                  stop=(grp_last[g] == (b, c)))
                      # group tail: aggT -> fp16, then per-window hn
                      aggT = stg.tile([P, GW], f16, tag="aggT")
                      if psw is not None:
                          nc.scalar.activation(aggT[:], psw[:], AF.Copy)
                      else:
                          nc.vector.memset(aggT[:], 0.0)
                      for wi in range(GRPW):
                          w = g * GRPW + wi
                          sl = slice(w * P, (w + 1) * P)
                          ph = ps_hn.tile([P, H], f32, tag="ph")
                          nc.tensor.matmul(ph[:], lhsT=aggT[:, wi * P:(wi + 1) * P],
                                           rhs=Wl_s[:], start=True, stop=False)
                          nc.tensor.matmul(ph[:], lhsT=hT[:, sl], rhs=Wr_s[:],
                                           start=False, stop=True)
                          hn = stg.tile([P, H], f32, tag="hn_s")
                          nc.vector.tensor_tensor(hn[:], ph[:], blb[:], OP.add)
                          stats = sml.tile([P, 6], f32, tag="st6")
                          mv = sml.tile([P, 2], f32, tag="mv")
                          nc.vector.bn_stats(stats[:], hn[:])
                          nc.vector.bn_aggr(mv[:], stats[:])
                          rstd = sml.tile([P, 1], f32, tag="rstd")
                          nc.scalar.activation(rstd[:], mv[:, 1:2], AF.Sqrt,
                                               bias=eps_c[:])
                          nc.vector.reciprocal(rstd[:], rstd[:])
                          t1 = stg.tile([P, H], f32, tag="t1")
                          nc.vector.scalar_tensor_tensor(
                              t1[:], hn[:], mv[:, 0:1], gb[:],
                              OP.subtract, OP.mult)
                          nc.vector.scalar_tensor_tensor(
                              t1[:], t1[:], rstd[:], bb[:], OP.mult, OP.add)
                          nc.scalar.activation(t1[:], t1[:], AF.Relu)
                          pt = ps_t.tile([P, P], f32, tag="pt")
                          nc.tensor.transpose(pt[:], hT[:, sl], ident_s[:])
                          hnew = stg.tile([P, H], f32, tag="hnew")
                          nc.vector.tensor_tensor(hnew[:], t1[:], pt[:], OP.add)
                          if l < L - 1:
                              h16 = stg.tile([P, H], f16, tag="h16")
                              nc.scalar.activation(h16[:], hnew[:], AF.Copy)
                              nc.sync.dma_start(h_shard[l + 1][sl, :], h16[:])
                          else:
                              # zero pad slots (h >= 0, so pooling sum/max
                              # over the padded window stays exact)
                              hm = stg.tile([P, H], f32, tag="hmask")
                              nc.scalar.activation(hm[:], hnew[:], AF.Copy,
                                                   scale=padmask_s[:, w:w + 1])
                              hnew = hm
                          pt2 = ps_t.tile([P, P], f32, tag="pt")
                          nc.tensor.transpose(pt2[:], hnew[:], ident_s[:])
                          nc.scalar.activation(hT[:, sl], pt2[:], AF.Copy)
                if l < L - 1 and "noag" not in flags:
                    nc.gpsimd.collective_compute(
                        "AllGather", OP.bypass, ins=[h_shard[l + 1].ap()],
                        outs=[h_rep[l + 1].ap()],
                        replica_groups=[list(range(NCORES))])

            # ---- pooling (hT holds final h; pad slots are zero) ----
            wsum = res.tile([P, W], f32)
            wmax = res.tile([P, W], f32)
            for w in range(W):
                sl = slice(w * P, (w + 1) * P)
                nc.vector.reduce_sum(wsum[:, w:w + 1], hT[:, sl],
                                     axis=mybir.AxisListType.X)
                nc.vector.reduce_max(wmax[:, w:w + 1], hT[:, sl],
                                     axis=mybir.AxisListType.X)
            gsum = sml.tile([P, GPC], f32, tag="gsum")
            gmax = sml.tile([P, GPC], f32, tag="gmax")
            for g in range(GPC):
                nc.vector.reduce_sum(gsum[:, g:g + 1], wsum[:, g * WG:(g + 1) * WG],
                                     axis=mybir.AxisListType.X)
                nc.vector.reduce_max(gmax[:, g:g + 1], wmax[:, g * WG:(g + 1) * WG],
                                     axis=mybir.AxisListType.X)
            icb = sml.tile([P, GPC], f32, tag="icb")
            nc.sync.dma_start(icb[:], bcast_row(invcnt_d[0, :], P, GPC))
            nc.vector.tensor_tensor(gsum[:], gsum[:], icb[:], OP.mult)

            # ---- trackster encoder (feature-major, GPC graphs) ----
            tsT_s = sml.tile([3, GPC], f32, tag="tsT")
            tsW1_s = sml.tile([3, HT], f32, tag="tsW1")
            tsW2_s = sml.tile([HT, HT], f32, tag="tsW2")
            tsb1_c = sml.tile([HT, 1], f32, tag="tsb1")
            tsb2_c = sml.tile([HT, 1], f32, tag="tsb2")
            nc.sync.dma_start(tsT_s[:], tsT_d[:])
            nc.sync.dma_start(tsW1_s[:], tsW1_d[:])
            nc.sync.dma_start(tsW2_s[:], tsW2_d[:])
            nc.sync.dma_start(tsb1_c[:], tsb1_d[:].rearrange("h -> h ()"))
            nc.sync.dma_start(tsb2_c[:], tsb2_d[:].rearrange("h -> h ()"))
            p1 = ps_hn.tile([HT, GPC], f32, tag="ph")
            nc.tensor.matmul(p1[:], lhsT=tsW1_s[:], rhs=tsT_s[:], start=True, stop=True)
            t1T = sml.tile([HT, GPC], f32, tag="t1T")
            nc.scalar.activation(t1T[:], p1[:], AF.Identity, bias=tsb1_c[:])
            # LN over HT in graph-major
            pg = ps_t.tile([GPC, HT], f32, tag="pt")
            nc.tensor.transpose(pg[:], t1T[:], ident_s[:HT, :HT])
            t1g = sml.tile([GPC, HT], f32, tag="t1g")
            nc.vector.tensor_copy(t1g[:], pg[:])
            tst = sml.tile([GPC, 6], f32, tag="tst6")
            tmv = sml.tile([GPC, 2], f32, tag="tsmv")
            nc.vector.bn_stats(tst[:], t1g[:])
            nc.vector.bn_aggr(tmv[:], tst[:])
            trs = sml.tile([GPC, 1], f32, tag="tsrstd")
            nc.scalar.activation(trs[:], tmv[:, 1:2], AF.Sqrt, bias=eps_c[:GPC, :])
            nc.vector.reciprocal(trs[:], trs[:])
            tlgb = sml.tile([GPC, HT], f32, tag="tlgb")
            tlbb = sml.tile([GPC, HT], f32, tag="tlbb")
            nc.sync.dma_start(tlgb[:], bcast_row(tslng_d[:], GPC, HT))
            nc.sync.dma_start(tlbb[:], bcast_row(tslnb_d[:], GPC, HT))
            nc.vector.scalar_tensor_tensor(t1g[:], t1g[:], tmv[:, 0:1], tlgb[:],
                                           OP.subtract, OP.mult)
            nc.vector.scalar_tensor_tensor(t1g[:], t1g[:], trs[:], tlbb[:],
                                           OP.mult, OP.add)
            nc.scalar.activation(t1g[:], t1g[:], AF.Relu)
            pr = ps_t.tile([HT, GPC], f32, tag="pt")
            nc.tensor.transpose(pr[:], t1g[:], ident_s[:GPC, :GPC])
            t1nT = sml.tile([HT, GPC], f32, tag="t1nT")
            nc.vector.tensor_copy(t1nT[:], pr[:])
            p2 = ps_hn.tile([HT, GPC], f32, tag="ph")
            nc.tensor.matmul(p2[:], lhsT=tsW2_s[:], rhs=t1nT[:], start=True, stop=True)
            t2T = sml.tile([HT, GPC], f32, tag="t2T")
            nc.scalar.activation(t2T[:], p2[:], AF.Identity, bias=tsb2_c[:])

            # ---- classifier ----
            PD = 2 * H + HT
            feat = sml.tile([GPC, PD], f32, tag="feat")
            pf = ps_t.tile([GPC, P], f32, tag="pt")
            nc.tensor.transpose(pf[:], gsum[:], ident_s[:])
            nc.vector.tensor_copy(feat[:, 0:H], pf[:])
            pf2 = ps_t.tile([GPC, P], f32, tag="pt")
            nc.tensor.transpose(pf2[:], gmax[:], ident_s[:])
            nc.vector.tensor_copy(feat[:, H:2 * H], pf2[:])
            pf3 = ps_t.tile([GPC, HT], f32, tag="pt")
            nc.tensor.transpose(pf3[:], t2T[:], ident_s[:HT, :HT])
            nc.vector.tensor_copy(feat[:, 2 * H:PD], pf3[:])
            # LN(PD)
            cst = sml.tile([GPC, 6], f32, tag="cst")
            cmv = sml.tile([GPC, 2], f32, tag="cmv")
            nc.vector.bn_stats(cst[:], feat[:])
            nc.vector.bn_aggr(cmv[:], cst[:])
            crs = sml.tile([GPC, 1], f32, tag="crs")
            nc.scalar.activation(crs[:], cmv[:, 1:2], AF.Sqrt, bias=eps_c[:GPC, :])
            nc.vector.reciprocal(crs[:], crs[:])
            cgb = sml.tile([GPC, PD], f32, tag="cgb")
            cbb = sml.tile([GPC, PD], f32, tag="cbb")
            nc.sync.dma_start(cgb[:], bcast_row(clng_d[:], GPC, PD))
            nc.sync.dma_start(cbb[:], bcast_row(clnb_d[:], GPC, PD))
            nc.vector.scalar_tensor_tensor(feat[:], feat[:], cmv[:, 0:1], cgb[:],
                                           OP.subtract, OP.mult)
            nc.vector.scalar_tensor_tensor(feat[:], feat[:], crs[:], cbb[:],
                                           OP.mult, OP.add)
            # z = relu(feat @ W1 + b1) in feature-major: zT [H, GPC]
            cb1_c = sml.tile([H, 1], f32, tag="cb1")
            nc.sync.dma_start(cb1_c[:], cb1_d[:].rearrange("h -> h ()"))
            pz = ps_hn.tile([H, GPC], f32, tag="ph")
            for j, (a, b_) in enumerate([(0, H), (H, 2 * H), (2 * H, PD)]):
                cW1j = sml.tile([b_ - a, H], f32, tag="cW1j", name=f"cW1j{j}")
                nc.sync.dma_start(cW1j[:], cW1_d[a:b_, :])
                pfj = ps_t.tile([b_ - a, GPC], f32, tag="pt")
                nc.tensor.transpose(pfj[:], feat[:, a:b_],
                                    ident_s[:GPC, :GPC])
                fTj = sml.tile([b_ - a, GPC], f32, tag="fTj")
                nc.vector.tensor_copy(fTj[:], pfj[:])
                nc.tensor.matmul(pz[:], lhsT=cW1j[:], rhs=fTj[:],
                                 start=(j == 0), stop=(j == 2))
            zT = sml.tile([H, GPC], f32, tag="zT")
            nc.scalar.activation(zT[:], pz[:], AF.Relu, bias=cb1_c[:])
            cW2_s = sml.tile([H, NCLS], f32, tag="cW2")
            nc.sync.dma_start(cW2_s[:], cW2_d[:])
            po = ps_hn.tile([GPC, NCLS], f32, tag="ph")
            nc.tensor.matmul(po[:], lhsT=zT[:], rhs=cW2_s[:], start=True, stop=True)
            ob = sml.tile([GPC, NCLS], f32, tag="ob")
            nc.sync.dma_start(ob[:], bcast_row(cb2_d[:], GPC, NCLS))
            outs = sml.tile([GPC, NCLS], f32, tag="outs")
            nc.vector.tensor_tensor(outs[:], po[:], ob[:], OP.add)
            nc.sync.dma_start(out_d[:], outs[:])

        for _rep in range(REPS):
            _pipeline()

    nc.compile()
    return nc


# ----------------------------------------------------------------------------
# entry point
# ----------------------------------------------------------------------------

def kernel(**inputs):
    from concourse.bass_utils import run_bass_kernel_spmd

    x = np.asarray(inputs["x"], np.float32)
    edge_index = np.asarray(inputs["edge_index"])
    batch = np.asarray(inputs["batch"])
    ts = np.asarray(inputs["ts"], np.float32)

    weights = {
        "enc_W": np.asarray(inputs["enc_W"], np.float32),
        "enc_b": np.asarray(inputs["enc_b"], np.float32),
        "sage_Wl": np.asarray(inputs["sage_Wl"], np.float16).reshape(L * H, H),
        "sage_bl": np.asarray(inputs["sage_bl"], np.float32),
        "sage_Wr": np.asarray(inputs["sage_Wr"], np.float32).reshape(L * H, H),
        "ln_g": np.asarray(inputs["ln_g"], np.float32),
        "ln_b": np.asarray(inputs["ln_b"], np.float32),
        "ts_W1": np.asarray(inputs["ts_W1"], np.float32),
        "ts_b1": np.asarray(inputs["ts_b1"], np.float32),
        "ts_lng": np.asarray(inputs["ts_lng"], np.float32),
        "ts_lnb": np.asarray(inputs["ts_lnb"], np.float32),
        "ts_W2": np.asarray(inputs["ts_W2"], np.float32),
        "ts_b2": np.asarray(inputs["ts_b2"], np.float32),
        "cls_lng": np.asarray(inputs["cls_lng"], np.float32),
        "cls_lnb": np.asarray(inputs["cls_lnb"], np.float32),
        "cls_W1": np.asarray(inputs["cls_W1"], np.float32),
        "cls_b1": np.asarray(inputs["cls_b1"], np.float32),
        "cls_W2": np.asarray(inputs["cls_W2"], np.float32),
        "cls_b2": np.asarray(inputs["cls_b2"], np.float32),
    }

    sched = _build_schedule(x, edge_index, batch)
    per_core = _host_inputs(sched, x, ts, weights)
    nc = _build_nc(sched)
    res = run_bass_kernel_spmd(nc, per_core, list(range(NCORES)), **_run_kwargs)
    if _res_hook is not None:
        _res_hook(res)
    return np.concatenate([res.results[c]["out"] for c in range(NCORES)], axis=0)


_run_kwargs = {}
_res_hook = None


# revision 19
# speedup vs baseline: 1.3156x; 1.3156x over previous
"""EnhancedGraphSAGE on 8 trn2 NeuronCores (Bass/Tile).

Sharding: 8 graphs per core (batch is sorted -> nodes graph-contiguous).
Each graph padded to G_slot slots (multiple of 128) with phantom nodes that
clone the graph's first node's features (no in-edges), so windows are
graph-pure with fully static shapes. h >= 0 always (relu encoder + relu
residual), so zeroing pad columns before pooling makes sum/max exact.
h is replicated across cores (fp16) via AllGather after the encoder and
after each SAGE layer.

Mean aggregation: per-core edges are grouped by (dst group of 4 windows,
src bank) into 128-edge chunks. dma_gather (int16 idx, 4 DRAM banks of the
fp16 replicated h, one SWDGE queue per bank) pulls h[src] rows into SBUF;
the PE accumulates aggT[f, slot-in-group] per 512-slot group as
gathered.T @ onehot where onehot[e, n] = (dlocal[e]==n) * invdeg[dst_e]
(fp16) is built on DVE with one fused tensor_scalar per chunk.
hn = agg@Wl + bl + h@Wr runs from aggT / resident hT (feature-major),
LN + relu + residual in node-major.
"""

import math
import os
from contextlib import ExitStack

import numpy as np

H = 128
HT = 64
NCLS = 8
L = 3
P = 128
NCORES = 8
GPC = 8  # graphs per core
GRPW = 4  # windows per psum group (512 dst slots)
GW = GRPW * P
SGG = 4  # groups per supergroup (one gather call per bank per supergroup)
MAX_BANK_ROWS = 32767
NBANKS = 4


# ----------------------------------------------------------------------------
# host-side schedule construction
# ----------------------------------------------------------------------------

def _build_schedule(x, edge_index, batch):
    N = x.shape[0]
    E = edge_index.shape[1]
    B = GPC * NCORES
    cnt = np.bincount(batch, minlength=B)
    assert cnt.min() > 0, "empty graph unsupported"
    gstart = np.zeros(B + 1, np.int64)
    np.cumsum(cnt, out=gstart[1:])
    G_slot = int(math.ceil(cnt.max() / P) * P)
    S = GPC * G_slot          # padded slots per core
    W = S // P                # windows per core
    WG = G_slot // P          # windows per graph
    assert S % GW == 0
    ngroups = S // GW
    bank_rows = int(math.ceil(NCORES * S / NBANKS))
    assert bank_rows <= MAX_BANK_ROWS

    g_of = batch.astype(np.int64)
    core_of_g = np.arange(B) // GPC
    slot_in_core_base = (np.arange(B) % GPC) * G_slot
    # per-core slot of real node n, and global replicated position
    slot = slot_in_core_base[g_of] + (np.arange(N) - gstart[g_of])
    p_rep = core_of_g[g_of] * S + slot

    src = edge_index[0].astype(np.int64)
    dst = edge_index[1].astype(np.int64)
    deg = np.bincount(dst, minlength=N).astype(np.float64)
    invdeg_node = 1.0 / np.maximum(deg, 1.0)

    e_core = core_of_g[g_of[dst]]
    e_slot = slot[dst]
    e_psrc = p_rep[src]
    e_inv = invdeg_node[dst]

    e_g = e_slot // GW        # dst group within core
    e_dl = e_slot % GW        # dst slot within group
    e_bank = e_psrc // bank_rows
    e_idx = e_psrc % bank_rows

    # per (core, group, bank) cell edge lists; chunk count = max over cores
    key = ((e_core * ngroups + e_g) * NBANKS + e_bank).astype(np.int64)
    korder = np.argsort(key, kind="stable")
    ks = key[korder]
    bounds = np.searchsorted(ks, np.arange(NCORES * ngroups * NBANKS + 1))

    def cell_list(c, g, b):
        k = (c * ngroups + g) * NBANKS + b
        return korder[bounds[k]:bounds[k + 1]]

    cell_chunks = np.zeros((ngroups, NBANKS), np.int64)
    for g in range(ngroups):
        for b in range(NBANKS):
            m = max(len(cell_list(c, g, b)) for c in range(NCORES))
            cell_chunks[g, b] = (m + P - 1) // P
    nchunks = int(cell_chunks.sum())

    # supergroups: SGG consecutive groups share one gather call per bank
    sgs = [list(range(s, min(s + SGG, ngroups))) for s in range(0, ngroups, SGG)]
    # chunk base of cell (g, b) within its (sg, b) call
    cell_base = {}
    call_nch = {}  # (si, b) -> chunks in call
    for si, sg in enumerate(sgs):
        for b in range(NBANKS):
            ofs = 0
            for g in sg:
                cell_base[(g, b)] = ofs
                ofs += int(cell_chunks[g, b])
            call_nch[(si, b)] = ofs

    # pack per-core idx / dlocal / invdegE in emission order
    # (each call's idx region 64B-aligned: 32 int16 cols)
    def _acols(nch):
        return -(-int(nch) * P // 16 // 32) * 32

    total_idx_cols = sum(_acols(call_nch[(si, b)])
                         for si in range(len(sgs)) for b in range(NBANKS))
    idx16 = np.zeros((NCORES, 128, total_idx_cols), np.int16)
    dlocal = np.full((NCORES, P, nchunks), 30000.0, np.float32)
    invdegE = np.zeros((NCORES, P, nchunks), np.float32)

    # global chunk offset of cell (g, b) in dlocal/invdegE, in emission order
    chunk_ofs = {}
    ofs = 0
    for si, sg in enumerate(sgs):
        for b in range(NBANKS):
            for g in sg:
                chunk_ofs[(g, b)] = ofs
                ofs += int(cell_chunks[g, b])
    assert ofs == nchunks

    for c in range(NCORES):
        colofs = 0
        for si, sg in enumerate(sgs):
            for b in range(NBANKS):
                nch_call = call_nch[(si, b)]
                if nch_call == 0:
                    continue
                vals = np.zeros(nch_call * P, np.int64)
                for g in sg:
                    nch = int(cell_chunks[g, b])
                    if nch == 0:
                        continue
                    lst = cell_list(c, g, b)
                    n = len(lst)
                    cb = cell_base[(g, b)]
                    vals[cb * P: cb * P + n] = e_idx[lst]
                    dlf = np.full(nch * P, 999.0, np.float32)
                    ivf = np.zeros(nch * P, np.float32)
                    dlf[:n] = e_dl[lst]
                    ivf[:n] = e_inv[lst]
                    co = chunk_ofs[(g, b)]
                    dlocal[c, :, co:co + nch] = dlf.reshape(nch, P).T
                    invdegE[c, :, co:co + nch] = ivf.reshape(nch, P).T
                ncols = nch_call * P // 16
                wrapped = vals.reshape(ncols, 16).T.astype(np.int16)
                for r in range(8):
                    idx16[c, r * 16:(r + 1) * 16, colofs:colofs + ncols] = wrapped
                colofs += _acols(nch_call)

    # first/last (bank, chunk) per group for PSUM start/stop
    grp_first = {}
    grp_last = {}
    for g in range(ngroups):
        nz = [(b, int(cell_chunks[g, b])) for b in range(NBANKS)
              if cell_chunks[g, b] > 0]
        if nz:
            grp_first[g] = (nz[0][0], 0)
            grp_last[g] = (nz[-1][0], nz[-1][1] - 1)

    return dict(
        N=N, E=E, B=B, cnt=cnt, gstart=gstart, G_slot=G_slot, S=S, W=W,
        WG=WG, bank_rows=bank_rows, slot=slot,
        cell_chunks=cell_chunks, nchunks=nchunks, sgs=sgs,
        cell_base=cell_base, call_nch=call_nch, chunk_ofs=chunk_ofs,
        idx16=idx16, dlocal=dlocal, invdegE=invdegE,
        total_idx_cols=total_idx_cols,
        grp_first=grp_first, grp_last=grp_last, ngroups=ngroups,
    )


def _host_inputs(sched, x, ts, weights):
    """Per-core input dicts (plus shared tensors replicated)."""
    S, G_slot = sched["S"], sched["G_slot"]
    cnt, gstart = sched["cnt"], sched["gstart"]
    slot = sched["slot"]

    xT = np.zeros((NCORES, 4, S), np.float32)
    g_all = np.repeat(np.arange(sched["B"]), cnt)
    for c in range(NCORES):
        sel = (g_all // GPC) == c
        xT[c, :, slot[sel]] = x[sel]
    # phantoms copy n0's features (keeps pad h finite; no in-edges)
    for g in range(sched["B"]):
        c = g // GPC
        base = (g % GPC) * G_slot
        nph = G_slot - cnt[g]
        if nph > 0:
            xT[c, :, base + cnt[g]: base + G_slot] = x[gstart[g]][:, None]

    invcnt = np.zeros((NCORES, GPC), np.float32)
    for g in range(sched["B"]):
        invcnt[g // GPC, g % GPC] = 1.0 / cnt[g]

    W = sched["W"]
    padmask = np.zeros((NCORES, P, W), np.float32)
    for g in range(sched["B"]):
        c = g // GPC
        base = (g % GPC) * G_slot
        real = np.arange(base, base + cnt[g])
        padmask[c, real % P, real // P] = 1.0

    iota512 = np.tile(np.arange(GW, dtype=np.float16), (P, 1))
    ident = np.eye(P, dtype=np.float32)

    per_core = []
    for c in range(NCORES):
        d = {
            "xT": np.ascontiguousarray(xT[c]),
            "gidx": np.ascontiguousarray(sched["idx16"][c]),
            "dlocal": np.ascontiguousarray(sched["dlocal"][c]),
            "invdegE": np.ascontiguousarray(sched["invdegE"][c]),
            "tsT": np.ascontiguousarray(ts[c * GPC:(c + 1) * GPC].T.astype(np.float32)),
            "invcnt": invcnt[c:c + 1],
            "padmask": np.ascontiguousarray(padmask[c]),
            "iota512": iota512,
            "ident": ident,
        }
        for k, v in weights.items():
            d[k] = v
        per_core.append(d)
    return per_core


# ----------------------------------------------------------------------------
# bass program
# ----------------------------------------------------------------------------

def _build_nc(sched):
    import concourse.bacc as bacc
    import concourse.bass as bass
    import concourse.mybir as mybir
    import concourse.tile as tile
    from concourse import library_config

    f32 = mybir.dt.float32
    f16 = mybir.dt.float16
    AF = mybir.ActivationFunctionType
    OP = mybir.AluOpType

    S, W, WG = sched["S"], sched["W"], sched["WG"]
    bank_rows = sched["bank_rows"]
    ngroups = sched["ngroups"]
    cell_chunks = sched["cell_chunks"]
    sgs = sched["sgs"]
    cell_base = sched["cell_base"]
    call_nch = sched["call_nch"]
    chunk_ofs = sched["chunk_ofs"]
    nchunks = sched["nchunks"]
    total_idx_cols = sched["total_idx_cols"]
    grp_first, grp_last = sched["grp_first"], sched["grp_last"]

    stage = os.environ.get("GNN_STAGE", "full")
    flags = set(stage.split("+"))
    nc = bacc.Bacc("TRN2", target_bir_lowering=False, num_swdge_queues=NBANKS)

    def din(name, shape, dtype=f32):
        return nc.dram_tensor(name, shape, dtype, kind="ExternalInput")

    xT_d = din("xT", [4, S])
    gidx_d = din("gidx", [128, total_idx_cols], mybir.dt.int16)
    dlocal_d = din("dlocal", [P, nchunks])
    invdegE_d = din("invdegE", [P, nchunks])
    tsT_d = din("tsT", [3, GPC])
    invcnt_d = din("invcnt", [1, GPC])
    padmask_d = din("padmask", [P, W])
    iota512_d = din("iota512", [P, GW], f16)
    ident_d = din("ident", [P, P])
    encW_d = din("enc_W", [4, H])
    encb_d = din("enc_b", [H])
    Wl_d = din("sage_Wl", [L * H, H], f16)
    bl_d = din("sage_bl", [L, H])
    Wr_d = din("sage_Wr", [L * H, H])
    lng_d = din("ln_g", [L, H])
    lnb_d = din("ln_b", [L, H])
    tsW1_d = din("ts_W1", [3, HT])
    tsb1_d = din("ts_b1", [HT])
    tslng_d = din("ts_lng", [HT])
    tslnb_d = din("ts_lnb", [HT])
    tsW2_d = din("ts_W2", [HT, HT])
    tsb2_d = din("ts_b2", [HT])
    clng_d = din("cls_lng", [2 * H + HT])
    clnb_d = din("cls_lnb", [2 * H + HT])
    cW1_d = din("cls_W1", [2 * H + HT, H])
    cb1_d = din("cls_b1", [H])
    cW2_d = din("cls_W2", [H, NCLS])
    cb2_d = din("cls_b2", [NCLS])
    out_d = nc.dram_tensor("out", [GPC, NCLS], f32, kind="ExternalOutput")

    h_shard = [nc.dram_tensor(f"h_shard{l}", [S, H], f16) for l in range(L)]
    h_rep = [nc.dram_tensor(f"h_rep{l}", [NCORES * S, H], f16,
                            addr_space="Shared") for l in range(L)]
    # one-hot cache: built on DVE in layer 0, streamed back in layers 1+
    # layout [partition, chunk*GW] so a run of chunks is one 2D DMA
    oh_dram = nc.dram_tensor("oh_cache", [P, nchunks * GW], f16)

    def bcast_row(dram_ap, npart, width):
        # AP reading a [width] or [1,width] dram row replicated across npart partitions
        return bass.AP(tensor=dram_ap.tensor, offset=dram_ap.offset,
                       ap=[[0, npart]] + dram_ap.ap[-1:])

    with tile.TileContext(nc) as tc, ExitStack() as ctx:
        res = ctx.enter_context(tc.tile_pool(name="res", bufs=1))
        gath = ctx.enter_context(tc.tile_pool(name="gath", bufs=5))
        oh = ctx.enter_context(tc.tile_pool(name="oh", bufs=3))
        stg = ctx.enter_context(tc.tile_pool(name="stg", bufs=4))
        sml = ctx.enter_context(tc.tile_pool(name="sml", bufs=2))
        ps_agg = ctx.enter_context(tc.tile_pool(name="ps_agg", bufs=2, space="PSUM"))
        ps_hn = ctx.enter_context(tc.tile_pool(name="ps_hn", bufs=2, space="PSUM"))
        ps_t = ctx.enter_context(tc.tile_pool(name="ps_t", bufs=2, space="PSUM"))

        nc.gpsimd.load_library(library_config.mlp)

        # ---- residents ----
        hT = res.tile([P, S], f32)                      # feature-major h shard
        gidx_s = res.tile([128, total_idx_cols], mybir.dt.int16)
        dl_s = res.tile([P, nchunks], f32)
        iv_s = res.tile([P, nchunks], f32)
        iota_s = res.tile([P, GW], f16)
        ident_s = res.tile([P, P], f32)
        encW_s = res.tile([4, H], f32)
        encb_c = res.tile([P, 1], f32)
        eps_c = res.tile([P, 1], f32)
        padmask_s = res.tile([P, W], f32)
        nc.sync.dma_start(padmask_s[:], padmask_d[:])
        nc.sync.dma_start(gidx_s[:], gidx_d[:])
        nc.sync.dma_start(dl_s[:], dlocal_d[:])
        nc.sync.dma_start(iv_s[:], invdegE_d[:])
        nc.sync.dma_start(iota_s[:], iota512_d[:])
        nc.sync.dma_start(ident_s[:], ident_d[:])
        nc.sync.dma_start(encW_s[:], encW_d[:])
        nc.sync.dma_start(encb_c[:], encb_d.ap().rearrange("h -> h ()"))
        nc.vector.memset(eps_c[:], 1e-5)

        REPS = int(os.environ.get("GNN_REPS", "1"))

        def _pipeline():
            # ---- encoder: hT = relu(enc_W.T @ xT + b) ----
            for w in range(W):
                sl = slice(w * P, (w + 1) * P)
                xw = stg.tile([4, P], f32, tag="xw")
                nc.sync.dma_start(xw[:], xT_d[:, sl])
                ps = ps_hn.tile([P, P], f32, tag="ph")
                nc.tensor.matmul(ps[:], lhsT=encW_s[:], rhs=xw[:],
                                 start=True, stop=True)
                nc.scalar.activation(hT[:, sl], ps[:], AF.Relu, bias=encb_c[:])
                pt = ps_t.tile([P, P], f32, tag="pt")
                nc.tensor.transpose(pt[:], hT[:, sl], ident_s[:])
                st = stg.tile([P, P], f16, tag="st16")
                nc.scalar.activation(st[:], pt[:], AF.Copy)
                nc.sync.dma_start(h_shard[0][sl, :], st[:])
            if not flags & {"noag", "nolayers"}:
                nc.gpsimd.collective_compute(
                    "AllGather", OP.bypass, ins=[h_shard[0].ap()],
                    outs=[h_rep[0].ap()], replica_groups=[list(range(NCORES))])

            # ---- SAGE layers ----
            for l in range(L if "nolayers" not in flags else 0):
                Wl_s = sml.tile([H, H], f16, tag="wl")
                Wr_s = sml.tile([H, H], f32, tag="wr")
                blb = sml.tile([P, H], f32, tag="blb")
                gb = sml.tile([P, H], f32, tag="gb")
                bb = sml.tile([P, H], f32, tag="bb")
                nc.sync.dma_start(Wl_s[:], Wl_d[l * H:(l + 1) * H, :])
                nc.sync.dma_start(Wr_s[:], Wr_d[l * H:(l + 1) * H, :])
                nc.sync.dma_start(blb[:], bcast_row(bl_d[l, :], P, H))
                nc.sync.dma_start(gb[:], bcast_row(lng_d[l, :], P, H))
                nc.sync.dma_start(bb[:], bcast_row(lnb_d[l, :], P, H))

                LREPS = int(os.environ.get("GNN_LREPS", "1"))
                for _lr in range(LREPS):
                  colofs = 0
                  for si, sg in enumerate(sgs):
                    gtiles = {}
                    for b in range(NBANKS):
                        nch_call = call_nch[(si, b)]
                        if nch_call == 0:
                            continue
                        ncols = nch_call * P // 16
                        acols = -(-ncols // 32) * 32
                        if "nogather" not in flags:
                            gt = gath.tile([P, nch_call, P], f16, tag="gath")
                            nc.gpsimd.dma_gather(
                                gt[:], h_rep[l][b * bank_rows:(b + 1) * bank_rows, :],
                                gidx_s[:, colofs:colofs + ncols],
                                nch_call * P, nch_call * P, H,
                                single_packet=(nch_call * P <= 1024),
                                queue_num=b)
                            gtiles[b] = gt
                        colofs += acols
                    for g in sg:
                      # chunk matmuls: bank-major, accumulating aggT per group
                      psw = None
                      if g in grp_first and not flags & {"nogather", "gatheronly"}:
                          psw = ps_agg.tile([P, GW], f32, tag="aggw")
                      for b in range(NBANKS):
                          if b not in gtiles or "gatheronly" in flags:
                              continue
                          assert "nogather" not in flags
                          nch = int(cell_chunks[g, b])
                          cb = cell_base[(g, b)]
                          co = chunk_ofs[(g, b)]
                          ohcell = oh.tile([P, nch, GW], f16, tag="ohc")
                          if l == 0 and _lr == 0:
                              for c in range(nch):
                                  nc.vector.tensor_scalar(
                                      ohcell[:, c, :], iota_s[:],
                                      dl_s[:, co + c:co + c + 1],
                                      iv_s[:, co + c:co + c + 1],
                                      OP.is_equal, OP.mult)
                              nc.sync.dma_start(
                                  oh_dram[:, co * GW:(co + nch) * GW],
                                  ohcell[:])
                          else:
                              nc.sync.dma_start(
                                  ohcell[:],
                                  oh_dram[:, co * GW:(co + nch) * GW])
                          for c in range(nch):
                              nc.tensor.matmul(
                                  psw[:], lhsT=gtiles[b][:, cb + c, :],
                                  rhs=ohcell[:, c, :],
                                  start=(grp_first[g] == (b, c)),
                                  stop=(grp_last[g] == (b, c)))
                      # group tail: aggT -> fp16, then per-window hn
                      aggT = stg.tile([P, GW], f16, tag="aggT")
                      if psw is not None:
                          nc.scalar.activation(aggT[:], psw[:], AF.Copy)
                      else:
                          nc.vector.memset(aggT[:], 0.0)
                      for wi in range(GRPW):
                          w = g * GRPW + wi
                          sl = slice(w * P, (w + 1) * P)
                          ph = ps_hn.tile([P, H], f32, tag="ph")
                          nc.tensor.matmul(ph[:], lhsT=aggT[:, wi * P:(wi + 1) * P],
                                           rhs=Wl_s[:], start=True, stop=False)
                          nc.tensor.matmul(ph[:], lhsT=hT[:, sl], rhs=Wr_s[:],
                                           start=False, stop=True)
                          hn = stg.tile([P, H], f32, tag="hn_s")
                          nc.vector.tensor_tensor(hn[:], ph[:], blb[:], OP.add)
                          stats = sml.tile([P, 6], f32, tag="st6")
                          mv = sml.tile([P, 2], f32, tag="mv")
                          nc.vector.bn_stats(stats[:], hn[:])
                          nc.vector.bn_aggr(mv[:], stats[:])
                          rstd = sml.tile([P, 1], f32, tag="rstd")
                          nc.scalar.activation(rstd[:], mv[:, 1:2], AF.Sqrt,
                                               bias=eps_c[:])
                          nc.vector.reciprocal(rstd[:], rstd[:])
                          t1 = stg.tile([P, H], f32, tag="t1")
                          nc.vector.scalar_tensor_tensor(
                              t1[:], hn[:], mv[:, 0:1], gb[:],
                              OP.subtract, OP.mult)
                          nc.vector.scalar_tensor_tensor(
                              t1[:], t1[:], rstd[:], bb[:], OP.mult, OP.add)
                          nc.scalar.activation(t1[:], t1[:], AF.Relu)
                          pt = ps_t.tile([P, P], f32, tag="pt")
                          nc.tensor.transpose(pt[:], hT[:, sl], ident_s[:])
                          hnew = stg.tile([P, H], f32, tag="hnew")
                          nc.vector.tensor_tensor(hnew[:], t1[:], pt[:], OP.add)
                          if l < L - 1:
                              h16 = stg.tile([P, H], f16, tag="h16")
                              nc.scalar.activation(h16[:], hnew[:], AF.Copy)
                              nc.sync.dma_start(h_shard[l + 1][sl, :], h16[:])
                          else:
                              # zero pad slots (h >= 0, so pooling sum/max
                              # over the padded window stays exact)
                              hm = stg.tile([P, H], f32, tag="hmask")
                              nc.scalar.activation(hm[:], hnew[:], AF.Copy,
                                                   scale=padmask_s[:, w:w + 1])
                              hnew = hm
                          pt2 = ps_t.tile([P, P], f32, tag="pt")
                          nc.tensor.transpose(pt2[:], hnew[:], ident_s[:])
                          nc.scalar.activation(hT[:, sl], pt2[:], AF.Copy)
                if l < L - 1 and "noag" not in flags:
                    nc.gpsimd.collective_compute(
                        "AllGather", OP.bypass, ins=[h_shard[l + 1].ap()],
                        outs=[h_rep[l + 1].ap()],
                        replica_groups=[list(range(NCORES))])

            # ---- pooling (hT holds final h; pad slots are zero) ----
            wsum = res.tile([P, W], f32)
            wmax = res.tile([P, W], f32)
            for w in range(W):
                sl = slice(w * P, (w + 1) * P)
                nc.vector.reduce_sum(wsum[:, w:w + 1], hT[:, sl],
                                     axis=mybir.AxisListType.X)
                nc.vector.reduce_max(wmax[:, w:w + 1], hT[:, sl],
                                     axis=mybir.AxisListType.X)
            gsum = sml.tile([P, GPC], f32, tag="gsum")
            gmax = sml.tile([P, GPC], f32, tag="gmax")
            for g in range(GPC):
                nc.vector.reduce_sum(gsum[:, g:g + 1], wsum[:, g * WG:(g + 1) * WG],
                                     axis=mybir.AxisListType.X)
                nc.vector.reduce_max(gmax[:, g:g + 1], wmax[:, g * WG:(g + 1) * WG],
                                     axis=mybir.AxisListType.X)
            icb = sml.tile([P, GPC], f32, tag="icb")
            nc.sync.dma_start(icb[:], bcast_row(invcnt_d[0, :], P, GPC))
            nc.vector.tensor_tensor(gsum[:], gsum[:], icb[:], OP.mult)

            # ---- trackster encoder (feature-major, GPC graphs) ----
            tsT_s = sml.tile([3, GPC], f32, tag="tsT")
            tsW1_s = sml.tile([3, HT], f32, tag="tsW1")
            tsW2_s = sml.tile([HT, HT], f32, tag="tsW2")
            tsb1_c = sml.tile([HT, 1], f32, tag="tsb1")
            tsb2_c = sml.tile([HT, 1], f32, tag="tsb2")
            nc.sync.dma_start(tsT_s[:], tsT_d[:])
            nc.sync.dma_start(tsW1_s[:], tsW1_d[:])
            nc.sync.dma_start(tsW2_s[:], tsW2_d[:])
            nc.sync.dma_start(tsb1_c[:], tsb1_d[:].rearrange("h -> h ()"))
            nc.sync.dma_start(tsb2_c[:], tsb2_d[:].rearrange("h -> h ()"))
            p1 = ps_hn.tile([HT, GPC], f32, tag="ph")
            nc.tensor.matmul(p1[:], lhsT=tsW1_s[:], rhs=tsT_s[:], start=True, stop=True)
            t1T = sml.tile([HT, GPC], f32, tag="t1T")
            nc.scalar.activation(t1T[:], p1[:], AF.Identity, bias=tsb1_c[:])
            # LN over HT in graph-major
            pg = ps_t.tile([GPC, HT], f32, tag="pt")
            nc.tensor.transpose(pg[:], t1T[:], ident_s[:HT, :HT])
            t1g = sml.tile([GPC, HT], f32, tag="t1g")
            nc.vector.tensor_copy(t1g[:], pg[:])
            tst = sml.tile([GPC, 6], f32, tag="tst6")
            tmv = sml.tile([GPC, 2], f32, tag="tsmv")
            nc.vector.bn_stats(tst[:], t1g[:])
            nc.vector.bn_aggr(tmv[:], tst[:])
            trs = sml.tile([GPC, 1], f32, tag="tsrstd")
            nc.scalar.activation(trs[:], tmv[:, 1:2], AF.Sqrt, bias=eps_c[:GPC, :])
            nc.vector.reciprocal(trs[:], trs[:])
            tlgb = sml.tile([GPC, HT], f32, tag="tlgb")
            tlbb = sml.tile([GPC, HT], f32, tag="tlbb")
            nc.sync.dma_start(tlgb[:], bcast_row(tslng_d[:], GPC, HT))
            nc.sync.dma_start(tlbb[:], bcast_row(tslnb_d[:], GPC, HT))
            nc.vector.scalar_tensor_tensor(t1g[:], t1g[:], tmv[:, 0:1], tlgb[:],
                                           OP.subtract, OP.mult)
            nc.vector.scalar_tensor_tensor(t1g[:], t1g[:], trs[:], tlbb[:],
                                           OP.mult, OP.add)
            nc.scalar.activation(t1g[:], t1g[:], AF.Relu)
            pr = ps_t.tile([HT, GPC], f32, tag="pt")
            nc.tensor.transpose(pr[:], t1g[:], ident_s[:GPC, :GPC])
            t1nT = sml.tile([HT, GPC], f32, tag="t1nT")
            nc.vector.tensor_copy(t1nT[:], pr[:])
            p2 = ps_hn.tile([HT, GPC], f32, tag="ph")
            nc.tensor.matmul(p2[:], lhsT=tsW2_s[:], rhs=t1nT[:], start=True, stop=True)
            t2T = sml.tile([HT, GPC], f32, tag="t2T")
            nc.scalar.activation(t2T[:], p2[:], AF.Identity, bias=tsb2_c[:])

            # ---- classifier ----
            PD = 2 * H + HT
            feat = sml.tile([GPC, PD], f32, tag="feat")
            pf = ps_t.tile([GPC, P], f32, tag="pt")
            nc.tensor.transpose(pf[:], gsum[:], ident_s[:])
            nc.vector.tensor_copy(feat[:, 0:H], pf[:])
            pf2 = ps_t.tile([GPC, P], f32, tag="pt")
            nc.tensor.transpose(pf2[:], gmax[:], ident_s[:])
            nc.vector.tensor_copy(feat[:, H:2 * H], pf2[:])
            pf3 = ps_t.tile([GPC, HT], f32, tag="pt")
            nc.tensor.transpose(pf3[:], t2T[:], ident_s[:HT, :HT])
            nc.vector.tensor_copy(feat[:, 2 * H:PD], pf3[:])
            # LN(PD)
            cst = sml.tile([GPC, 6], f32, tag="cst")
            cmv = sml.tile([GPC, 2], f32, tag="cmv")
            nc.vector.bn_stats(cst[:], feat[:])
            nc.vector.bn_aggr(cmv[:], cst[:])
            crs = sml.tile([GPC, 1], f32, tag="crs")
            nc.scalar.activation(crs[:], cmv[:, 1:2], AF.Sqrt, bias=eps_c[:GPC, :])
            nc.vector.reciprocal(crs[:], crs[:])
            cgb = sml.tile([GPC, PD], f32, tag="cgb")
            cbb = sml.tile([GPC, PD], f32, tag="cbb")
            nc.sync.dma_start(cgb[:], bcast_row(clng_d[:], GPC, PD))
            nc.sync.dma_start(cbb[:], bcast_row(clnb_d[:], GPC, PD))
            nc.vector.scalar_tensor_tensor(feat[:], feat[:], cmv[:, 0:1], cgb[:],
                                           OP.subtract, OP.mult)
            nc.vector.scalar_tensor_tensor(feat[:], feat[:], crs[:], cbb[:],
                                           OP.mult, OP.add)
            # z = relu(feat @ W1 + b1) in feature-major: zT [H, GPC]
            cb1_c = sml.tile([H, 1], f32, tag="cb1")
            nc.sync.dma_start(cb1_c[:], cb1_d[:].rearrange("h -> h ()"))
            pz = ps_hn.tile([H, GPC], f32, tag="ph")
            for j, (a, b_) in enumerate([(0, H), (H, 2 * H), (2 * H, PD)]):
                cW1j = sml.tile([b_ - a, H], f32, tag="cW1j", name=f"cW1j{j}")
                nc.sync.dma_start(cW1j[:], cW1_d[a:b_, :])
                pfj = ps_t.tile([b_ - a, GPC], f32, tag="pt")
                nc.tensor.transpose(pfj[:], feat[:, a:b_],
                                    ident_s[:GPC, :GPC])
                fTj = sml.tile([b_ - a, GPC], f32, tag="fTj")
                nc.vector.tensor_copy(fTj[:], pfj[:])
                nc.tensor.matmul(pz[:], lhsT=cW1j[:], rhs=fTj[:],
                                 start=(j == 0), stop=(j == 2))
            zT = sml.tile([H, GPC], f32, tag="zT")
            nc.scalar.activation(zT[:], pz[:], AF.Relu, bias=cb1_c[:])
            cW2_s = sml.tile([H, NCLS], f32, tag="cW2")
            nc.sync.dma_start(cW2_s[:], cW2_d[:])
            po = ps_hn.tile([GPC, NCLS], f32, tag="ph")
            nc.tensor.matmul(po[:], lhsT=zT[:], rhs=cW2_s[:], start=True, stop=True)
            ob = sml.tile([GPC, NCLS], f32, tag="ob")
            nc.sync.dma_start(ob[:], bcast_row(cb2_d[:], GPC, NCLS))
            outs = sml.tile([GPC, NCLS], f32, tag="outs")
            nc.vector.tensor_tensor(outs[:], po[:], ob[:], OP.add)
            nc.sync.dma_start(out_d[:], outs[:])

        for _rep in range(REPS):
            _pipeline()

    nc.compile()
    return nc


# ----------------------------------------------------------------------------
# entry point
# ----------------------------------------------------------------------------

def kernel(**inputs):
    from concourse.bass_utils import run_bass_kernel_spmd

    x = np.asarray(inputs["x"], np.float32)
    edge_index = np.asarray(inputs["edge_index"])
    batch = np.asarray(inputs["batch"])
    ts = np.asarray(inputs["ts"], np.float32)

    weights = {
        "enc_W": np.asarray(inputs["enc_W"], np.float32),
        "enc_b": np.asarray(inputs["enc_b"], np.float32),
        "sage_Wl": np.asarray(inputs["sage_Wl"], np.float16).reshape(L * H, H),
        "sage_bl": np.asarray(inputs["sage_bl"], np.float32),
        "sage_Wr": np.asarray(inputs["sage_Wr"], np.float32).reshape(L * H, H),
        "ln_g": np.asarray(inputs["ln_g"], np.float32),
        "ln_b": np.asarray(inputs["ln_b"], np.float32),
        "ts_W1": np.asarray(inputs["ts_W1"], np.float32),
        "ts_b1": np.asarray(inputs["ts_b1"], np.float32),
        "ts_lng": np.asarray(inputs["ts_lng"], np.float32),
        "ts_lnb": np.asarray(inputs["ts_lnb"], np.float32),
        "ts_W2": np.asarray(inputs["ts_W2"], np.float32),
        "ts_b2": np.asarray(inputs["ts_b2"], np.float32),
        "cls_lng": np.asarray(inputs["cls_lng"], np.float32),
        "cls_lnb": np.asarray(inputs["cls_lnb"], np.float32),
        "cls_W1": np.asarray(inputs["cls_W1"], np.float32),
        "cls_b1": np.asarray(inputs["cls_b1"], np.float32),
        "cls_W2": np.asarray(inputs["cls_W2"], np.float32),
        "cls_b2": np.asarray(inputs["cls_b2"], np.float32),
    }

    sched = _build_schedule(x, edge_index, batch)
    per_core = _host_inputs(sched, x, ts, weights)
    nc = _build_nc(sched)
    res = run_bass_kernel_spmd(nc, per_core, list(range(NCORES)), **_run_kwargs)
    if _res_hook is not None:
        _res_hook(res)
    return np.concatenate([res.results[c]["out"] for c in range(NCORES)], axis=0)


_run_kwargs = {}
_res_hook = None


# revision 22
# speedup vs baseline: 1.5492x; 1.1776x over previous
"""EnhancedGraphSAGE on 8 trn2 NeuronCores (Bass/Tile).

Sharding: 8 graphs per core (batch is sorted -> nodes graph-contiguous).
Each graph padded to G_slot slots (multiple of 128) with phantom nodes that
clone the graph's first node's features (no in-edges), so windows are
graph-pure with fully static shapes. h >= 0 always (relu encoder + relu
residual), so zeroing pad columns before pooling makes sum/max exact.
h is replicated across cores (fp16) via AllGather after the encoder and
after each SAGE layer.

Mean aggregation: per-core edges are grouped by (dst group of 4 windows,
src bank) into 128-edge chunks. dma_gather (int16 idx, 4 DRAM banks of the
fp16 replicated h, one SWDGE queue per bank) pulls h[src] rows into SBUF;
the PE accumulates aggT[f, slot-in-group] per 512-slot group as
gathered.T @ onehot where onehot[e, n] = (dlocal[e]==n) * invdeg[dst_e]
(fp16) is built on DVE with one fused tensor_scalar per chunk.
hn = agg@Wl + bl + h@Wr runs from aggT / resident hT (feature-major),
LN + relu + residual in node-major.
"""

import math
import os
from contextlib import ExitStack

import numpy as np

H = 128
HT = 64
NCLS = 8
L = 3
P = 128
NCORES = 8
GPC = 8  # graphs per core
GRPW = 4  # windows per psum group (512 dst slots)
GW = GRPW * P
SGG = 2  # groups per supergroup (one gather call per bank per supergroup)
MAX_BANK_ROWS = 32767
NBANKS = 4


# ----------------------------------------------------------------------------
# host-side schedule construction
# ----------------------------------------------------------------------------

def _build_schedule(x, edge_index, batch):
    N = x.shape[0]
    E = edge_index.shape[1]
    B = GPC * NCORES
    cnt = np.bincount(batch, minlength=B)
    assert cnt.min() > 0, "empty graph unsupported"
    gstart = np.zeros(B + 1, np.int64)
    np.cumsum(cnt, out=gstart[1:])
    G_slot = int(math.ceil(cnt.max() / P) * P)
    S = GPC * G_slot          # padded slots per core
    W = S // P                # windows per core
    WG = G_slot // P          # windows per graph
    assert S % GW == 0
    ngroups = S // GW
    bank_rows = int(math.ceil(NCORES * S / NBANKS))
    assert bank_rows <= MAX_BANK_ROWS

    g_of = batch.astype(np.int64)
    core_of_g = np.arange(B) // GPC
    slot_in_core_base = (np.arange(B) % GPC) * G_slot
    # per-core slot of real node n
    slot = slot_in_core_base[g_of] + (np.arange(N) - gstart[g_of])
    core_of_n = core_of_g[g_of]

    src = edge_index[0].astype(np.int64)
    dst = edge_index[1].astype(np.int64)
    deg = np.bincount(dst, minlength=N).astype(np.float64)
    invdeg_node = 1.0 / np.maximum(deg, 1.0)

    # bank = quarter of the per-core slot space; quarter q of every core's
    # shard is AllGathered into h_rep rows [q*bank_rows, (q+1)*bank_rows)
    # as [core, slot-in-quarter], so bank b only depends on collective b.
    SQ = S // NBANKS
    assert bank_rows == NCORES * SQ

    e_core = core_of_g[g_of[dst]]
    e_slot = slot[dst]
    e_inv = invdeg_node[dst]

    e_g = e_slot // GW        # dst group within core
    e_dl = e_slot % GW        # dst slot within group
    e_bank = slot[src] // SQ
    e_idx = core_of_n[src] * SQ + (slot[src] % SQ)

    # per (core, group, bank) cell edge lists; chunk count = max over cores
    key = ((e_core * ngroups + e_g) * NBANKS + e_bank).astype(np.int64)
    korder = np.argsort(key, kind="stable")
    ks = key[korder]
    bounds = np.searchsorted(ks, np.arange(NCORES * ngroups * NBANKS + 1))

    def cell_list(c, g, b):
        k = (c * ngroups + g) * NBANKS + b
        return korder[bounds[k]:bounds[k + 1]]

    cell_chunks = np.zeros((ngroups, NBANKS), np.int64)
    for g in range(ngroups):
        for b in range(NBANKS):
            m = max(len(cell_list(c, g, b)) for c in range(NCORES))
            cell_chunks[g, b] = (m + P - 1) // P
    nchunks = int(cell_chunks.sum())

    # supergroups: SGG consecutive groups share one gather call per bank
    sgs = [list(range(s, min(s + SGG, ngroups))) for s in range(0, ngroups, SGG)]
    # chunk base of cell (g, b) within its (sg, b) call
    cell_base = {}
    call_nch = {}  # (si, b) -> chunks in call
    for si, sg in enumerate(sgs):
        for b in range(NBANKS):
            ofs = 0
            for g in sg:
                cell_base[(g, b)] = ofs
                ofs += int(cell_chunks[g, b])
            call_nch[(si, b)] = ofs

    # pack per-core idx / dlocal / invdegE in emission order
    # (each call's idx region 64B-aligned: 32 int16 cols)
    def _acols(nch):
        return -(-int(nch) * P // 16 // 32) * 32

    total_idx_cols = sum(_acols(call_nch[(si, b)])
                         for si in range(len(sgs)) for b in range(NBANKS))
    idx16 = np.zeros((NCORES, 128, total_idx_cols), np.int16)
    dlocal = np.full((NCORES, P, nchunks), 30000.0, np.float32)
    invdegE = np.zeros((NCORES, P, nchunks), np.float32)

    # global chunk offset of cell (g, b) in dlocal/invdegE, in emission order
    chunk_ofs = {}
    ofs = 0
    for si, sg in enumerate(sgs):
        for b in range(NBANKS):
            for g in sg:
                chunk_ofs[(g, b)] = ofs
                ofs += int(cell_chunks[g, b])
    assert ofs == nchunks

    for c in range(NCORES):
        colofs = 0
        for si, sg in enumerate(sgs):
            for b in range(NBANKS):
                nch_call = call_nch[(si, b)]
                if nch_call == 0:
                    continue
                vals = np.zeros(nch_call * P, np.int64)
                for g in sg:
                    nch = int(cell_chunks[g, b])
                    if nch == 0:
                        continue
                    lst = cell_list(c, g, b)
                    n = len(lst)
                    cb = cell_base[(g, b)]
                    vals[cb * P: cb * P + n] = e_idx[lst]
                    dlf = np.full(nch * P, 999.0, np.float32)
                    ivf = np.zeros(nch * P, np.float32)
                    dlf[:n] = e_dl[lst]
                    ivf[:n] = e_inv[lst]
                    co = chunk_ofs[(g, b)]
                    dlocal[c, :, co:co + nch] = dlf.reshape(nch, P).T
                    invdegE[c, :, co:co + nch] = ivf.reshape(nch, P).T
                ncols = nch_call * P // 16
                wrapped = vals.reshape(ncols, 16).T.astype(np.int16)
                for r in range(8):
                    idx16[c, r * 16:(r + 1) * 16, colofs:colofs + ncols] = wrapped
                colofs += _acols(nch_call)

    # first/last (bank, chunk) per group for PSUM start/stop
    grp_first = {}
    grp_last = {}
    for g in range(ngroups):
        nz = [(b, int(cell_chunks[g, b])) for b in range(NBANKS)
              if cell_chunks[g, b] > 0]
        if nz:
            grp_first[g] = (nz[0][0], 0)
            grp_last[g] = (nz[-1][0], nz[-1][1] - 1)

    return dict(
        N=N, E=E, B=B, cnt=cnt, gstart=gstart, G_slot=G_slot, S=S, W=W,
        WG=WG, bank_rows=bank_rows, slot=slot, SQ=SQ,
        cell_chunks=cell_chunks, nchunks=nchunks, sgs=sgs,
        cell_base=cell_base, call_nch=call_nch, chunk_ofs=chunk_ofs,
        idx16=idx16, dlocal=dlocal, invdegE=invdegE,
        total_idx_cols=total_idx_cols,
        grp_first=grp_first, grp_last=grp_last, ngroups=ngroups,
    )


def _host_inputs(sched, x, ts, weights):
    """Per-core input dicts (plus shared tensors replicated)."""
    S, G_slot = sched["S"], sched["G_slot"]
    cnt, gstart = sched["cnt"], sched["gstart"]
    slot = sched["slot"]

    xT = np.zeros((NCORES, 4, S), np.float32)
    g_all = np.repeat(np.arange(sched["B"]), cnt)
    for c in range(NCORES):
        sel = (g_all // GPC) == c
        xT[c, :, slot[sel]] = x[sel]
    # phantoms copy n0's features (keeps pad h finite; no in-edges)
    for g in range(sched["B"]):
        c = g // GPC
        base = (g % GPC) * G_slot
        nph = G_slot - cnt[g]
        if nph > 0:
            xT[c, :, base + cnt[g]: base + G_slot] = x[gstart[g]][:, None]

    invcnt = np.zeros((NCORES, GPC), np.float32)
    for g in range(sched["B"]):
        invcnt[g // GPC, g % GPC] = 1.0 / cnt[g]

    W = sched["W"]
    padmask = np.zeros((NCORES, P, W), np.float32)
    for g in range(sched["B"]):
        c = g // GPC
        base = (g % GPC) * G_slot
        real = np.arange(base, base + cnt[g])
        padmask[c, real % P, real // P] = 1.0

    iota512 = np.tile(np.arange(GW, dtype=np.float16), (P, 1))
    ident = np.eye(P, dtype=np.float32)

    per_core = []
    for c in range(NCORES):
        d = {
            "xT": np.ascontiguousarray(xT[c]),
            "gidx": np.ascontiguousarray(sched["idx16"][c]),
            "dlocal": np.ascontiguousarray(sched["dlocal"][c]),
            "invdegE": np.ascontiguousarray(sched["invdegE"][c]),
            "tsT": np.ascontiguousarray(ts[c * GPC:(c + 1) * GPC].T.astype(np.float32)),
            "invcnt": invcnt[c:c + 1],
            "padmask": np.ascontiguousarray(padmask[c]),
            "iota512": iota512,
            "ident": ident,
        }
        for k, v in weights.items():
            d[k] = v
        per_core.append(d)
    return per_core


# ----------------------------------------------------------------------------
# bass program
# ----------------------------------------------------------------------------

def _build_nc(sched):
    import concourse.bacc as bacc
    import concourse.bass as bass
    import concourse.mybir as mybir
    import concourse.tile as tile
    from concourse import library_config

    f32 = mybir.dt.float32
    f16 = mybir.dt.float16
    AF = mybir.ActivationFunctionType
    OP = mybir.AluOpType

    S, W, WG = sched["S"], sched["W"], sched["WG"]
    SQ = sched["SQ"]
    WQ = W // NBANKS
    bank_rows = sched["bank_rows"]
    ngroups = sched["ngroups"]
    cell_chunks = sched["cell_chunks"]
    sgs = sched["sgs"]
    cell_base = sched["cell_base"]
    call_nch = sched["call_nch"]
    chunk_ofs = sched["chunk_ofs"]
    nchunks = sched["nchunks"]
    total_idx_cols = sched["total_idx_cols"]
    grp_first, grp_last = sched["grp_first"], sched["grp_last"]

    stage = os.environ.get("GNN_STAGE", "full")
    flags = set(stage.split("+"))
    nc = bacc.Bacc("TRN2", target_bir_lowering=False, num_swdge_queues=NBANKS)

    def din(name, shape, dtype=f32):
        return nc.dram_tensor(name, shape, dtype, kind="ExternalInput")

    xT_d = din("xT", [4, S])
    gidx_d = din("gidx", [128, total_idx_cols], mybir.dt.int16)
    dlocal_d = din("dlocal", [P, nchunks])
    invdegE_d = din("invdegE", [P, nchunks])
    tsT_d = din("tsT", [3, GPC])
    invcnt_d = din("invcnt", [1, GPC])
    padmask_d = din("padmask", [P, W])
    iota512_d = din("iota512", [P, GW], f16)
    ident_d = din("ident", [P, P])
    encW_d = din("enc_W", [4, H])
    encb_d = din("enc_b", [H])
    Wl_d = din("sage_Wl", [L * H, H], f16)
    bl_d = din("sage_bl", [L, H])
    Wr_d = din("sage_Wr", [L * H, H])
    lng_d = din("ln_g", [L, H])
    lnb_d = din("ln_b", [L, H])
    tsW1_d = din("ts_W1", [3, HT])
    tsb1_d = din("ts_b1", [HT])
    tslng_d = din("ts_lng", [HT])
    tslnb_d = din("ts_lnb", [HT])
    tsW2_d = din("ts_W2", [HT, HT])
    tsb2_d = din("ts_b2", [HT])
    clng_d = din("cls_lng", [2 * H + HT])
    clnb_d = din("cls_lnb", [2 * H + HT])
    cW1_d = din("cls_W1", [2 * H + HT, H])
    cb1_d = din("cls_b1", [H])
    cW2_d = din("cls_W2", [H, NCLS])
    cb2_d = din("cls_b2", [NCLS])
    out_d = nc.dram_tensor("out", [GPC, NCLS], f32, kind="ExternalOutput")

    h_shard = [nc.dram_tensor(f"h_shard{l}", [S, H], f16) for l in range(L)]
    h_rep = [nc.dram_tensor(f"h_rep{l}", [NCORES * S, H], f16,
                            addr_space="Shared") for l in range(L)]
    # one-hot cache: built on DVE in layer 0, streamed back in layers 1+
    # layout [partition, chunk*GW] so a run of chunks is one 2D DMA
    oh_dram = nc.dram_tensor("oh_cache", [P, nchunks * GW], f16)

    def bcast_row(dram_ap, npart, width):
        # AP reading a [width] or [1,width] dram row replicated across npart partitions
        return bass.AP(tensor=dram_ap.tensor, offset=dram_ap.offset,
                       ap=[[0, npart]] + dram_ap.ap[-1:])

    with tile.TileContext(nc) as tc, ExitStack() as ctx:
        res = ctx.enter_context(tc.tile_pool(name="res", bufs=1))
        gath = ctx.enter_context(tc.tile_pool(name="gath", bufs=8))
        oh = ctx.enter_context(tc.tile_pool(name="oh", bufs=4))
        stg = ctx.enter_context(tc.tile_pool(name="stg", bufs=4))
        sml = ctx.enter_context(tc.tile_pool(name="sml", bufs=2))
        ps_agg = ctx.enter_context(tc.tile_pool(name="ps_agg", bufs=2, space="PSUM"))
        ps_hn = ctx.enter_context(tc.tile_pool(name="ps_hn", bufs=2, space="PSUM"))
        ps_t = ctx.enter_context(tc.tile_pool(name="ps_t", bufs=2, space="PSUM"))

        nc.gpsimd.load_library(library_config.mlp)

        # ---- residents ----
        hT = res.tile([P, S], f32)                      # feature-major h shard
        gidx_s = res.tile([128, total_idx_cols], mybir.dt.int16)
        dl_s = res.tile([P, nchunks], f32)
        iv_s = res.tile([P, nchunks], f32)
        iota_s = res.tile([P, GW], f16)
        ident_s = res.tile([P, P], f32)
        encW_s = res.tile([4, H], f32)
        encb_c = res.tile([P, 1], f32)
        eps_c = res.tile([P, 1], f32)
        padmask_s = res.tile([P, W], f32)
        nc.sync.dma_start(padmask_s[:], padmask_d[:])
        nc.sync.dma_start(gidx_s[:], gidx_d[:])
        nc.sync.dma_start(dl_s[:], dlocal_d[:])
        nc.sync.dma_start(iv_s[:], invdegE_d[:])
        nc.sync.dma_start(iota_s[:], iota512_d[:])
        nc.sync.dma_start(ident_s[:], ident_d[:])
        nc.sync.dma_start(encW_s[:], encW_d[:])
        nc.sync.dma_start(encb_c[:], encb_d.ap().rearrange("h -> h ()"))
        nc.vector.memset(eps_c[:], 1e-5)

        REPS = int(os.environ.get("GNN_REPS", "1"))

        def _pipeline():
            # ---- encoder: hT = relu(enc_W.T @ xT + b) ----
            for w in range(W):
                sl = slice(w * P, (w + 1) * P)
                xw = stg.tile([4, P], f32, tag="xw")
                nc.sync.dma_start(xw[:], xT_d[:, sl])
                ps = ps_hn.tile([P, P], f32, tag="ph")
                nc.tensor.matmul(ps[:], lhsT=encW_s[:], rhs=xw[:],
                                 start=True, stop=True)
                nc.scalar.activation(hT[:, sl], ps[:], AF.Relu, bias=encb_c[:])
                pt = ps_t.tile([P, P], f32, tag="pt")
                nc.tensor.transpose(pt[:], hT[:, sl], ident_s[:])
                st = stg.tile([P, P], f16, tag="st16")
                nc.scalar.activation(st[:], pt[:], AF.Copy)
                nc.sync.dma_start(h_shard[0][sl, :], st[:])
                if (w + 1) % WQ == 0 and not flags & {"noag", "nolayers"}:
                    q = w // WQ
                    nc.gpsimd.collective_compute(
                        "AllGather", OP.bypass,
                        ins=[h_shard[0][q * SQ:(q + 1) * SQ, :]],
                        outs=[h_rep[0][q * bank_rows:(q + 1) * bank_rows, :]],
                        replica_groups=[list(range(NCORES))])

            # ---- SAGE layers ----
            for l in range(L if "nolayers" not in flags else 0):
                Wl_s = sml.tile([H, H], f16, tag="wl")
                Wr_s = sml.tile([H, H], f32, tag="wr")
                blb = sml.tile([P, H], f32, tag="blb")
                gb = sml.tile([P, H], f32, tag="gb")
                bb = sml.tile([P, H], f32, tag="bb")
                nc.sync.dma_start(Wl_s[:], Wl_d[l * H:(l + 1) * H, :])
                nc.sync.dma_start(Wr_s[:], Wr_d[l * H:(l + 1) * H, :])
                nc.sync.dma_start(blb[:], bcast_row(bl_d[l, :], P, H))
                nc.sync.dma_start(gb[:], bcast_row(lng_d[l, :], P, H))
                nc.sync.dma_start(bb[:], bcast_row(lnb_d[l, :], P, H))

                LREPS = int(os.environ.get("GNN_LREPS", "1"))
                for _lr in range(LREPS):
                  colofs = 0
                  for si, sg in enumerate(sgs):
                    gtiles = {}
                    for b in range(NBANKS):
                        nch_call = call_nch[(si, b)]
                        if nch_call == 0:
                            continue
                        ncols = nch_call * P // 16
                        acols = -(-ncols // 32) * 32
                        if "nogather" not in flags:
                            gt = gath.tile([P, nch_call, P], f16, tag="gath")
                            nc.gpsimd.dma_gather(
                                gt[:], h_rep[l][b * bank_rows:(b + 1) * bank_rows, :],
                                gidx_s[:, colofs:colofs + ncols],
                                nch_call * P, nch_call * P, H,
                                single_packet=(nch_call * P <= 1024),
                                queue_num=b)
                            gtiles[b] = gt
                        colofs += acols
                    for g in sg:
                      # chunk matmuls: bank-major, accumulating aggT per group
                      psw = None
                      if g in grp_first and not flags & {"nogather", "gatheronly"}:
                          psw = ps_agg.tile([P, GW], f32, tag="aggw")
                      for b in range(NBANKS):
                          if b not in gtiles or "gatheronly" in flags:
                              continue
                          assert "nogather" not in flags
                          nch = int(cell_chunks[g, b])
                          cb = cell_base[(g, b)]
                          co = chunk_ofs[(g, b)]
                          ohcell = oh.tile([P, nch, GW], f16, tag="ohc")
                          if l == 0 and _lr == 0:
                              for c in range(nch):
                                  nc.vector.tensor_scalar(
                                      ohcell[:, c, :], iota_s[:],
                                      dl_s[:, co + c:co + c + 1],
                                      iv_s[:, co + c:co + c + 1],
                                      OP.is_equal, OP.mult)
                              nc.sync.dma_start(
                                  oh_dram[:, co * GW:(co + nch) * GW],
                                  ohcell[:])
                          else:
                              nc.sync.dma_start(
                                  ohcell[:],
                                  oh_dram[:, co * GW:(co + nch) * GW])
                          for c in range(nch):
                              nc.tensor.matmul(
                                  psw[:], lhsT=gtiles[b][:, cb + c, :],
                                  rhs=ohcell[:, c, :],
                                  start=(grp_first[g] == (b, c)),
                                  stop=(grp_last[g] == (b, c)))
                      # group tail: aggT -> fp16, then per-window hn
                      aggT = stg.tile([P, GW], f16, tag="aggT")
                      if psw is not None:
                          nc.scalar.activation(aggT[:], psw[:], AF.Copy)
                      else:
                          nc.vector.memset(aggT[:], 0.0)
                      for wi in range(GRPW):
                          w = g * GRPW + wi
                          sl = slice(w * P, (w + 1) * P)
                          ph = ps_hn.tile([P, H], f32, tag="ph")
                          nc.tensor.matmul(ph[:], lhsT=aggT[:, wi * P:(wi + 1) * P],
                                           rhs=Wl_s[:], start=True, stop=False)
                          nc.tensor.matmul(ph[:], lhsT=hT[:, sl], rhs=Wr_s[:],
                                           start=False, stop=True)
                          hn = stg.tile([P, H], f32, tag="hn_s")
                          nc.vector.tensor_tensor(hn[:], ph[:], blb[:], OP.add)
                          stats = sml.tile([P, 6], f32, tag="st6")
                          mv = sml.tile([P, 2], f32, tag="mv")
                          nc.vector.bn_stats(stats[:], hn[:])
                          nc.vector.bn_aggr(mv[:], stats[:])
                          rstd = sml.tile([P, 1], f32, tag="rstd")
                          nc.scalar.activation(rstd[:], mv[:, 1:2], AF.Sqrt,
                                               bias=eps_c[:])
                          nc.vector.reciprocal(rstd[:], rstd[:])
                          t1 = stg.tile([P, H], f32, tag="t1")
                          nc.vector.scalar_tensor_tensor(
                              t1[:], hn[:], mv[:, 0:1], gb[:],
                              OP.subtract, OP.mult)
                          nc.vector.scalar_tensor_tensor(
                              t1[:], t1[:], rstd[:], bb[:], OP.mult, OP.add)
                          nc.scalar.activation(t1[:], t1[:], AF.Relu)
                          pt = ps_t.tile([P, P], f32, tag="pt")
                          nc.tensor.transpose(pt[:], hT[:, sl], ident_s[:])
                          hnew = stg.tile([P, H], f32, tag="hnew")
                          nc.vector.tensor_tensor(hnew[:], t1[:], pt[:], OP.add)
                          if l < L - 1:
                              h16 = stg.tile([P, H], f16, tag="h16")
                              nc.scalar.activation(h16[:], hnew[:], AF.Copy)
                              nc.sync.dma_start(h_shard[l + 1][sl, :], h16[:])
                          else:
                              # zero pad slots (h >= 0, so pooling sum/max
                              # over the padded window stays exact)
                              hm = stg.tile([P, H], f32, tag="hmask")
                              nc.scalar.activation(hm[:], hnew[:], AF.Copy,
                                                   scale=padmask_s[:, w:w + 1])
                              hnew = hm
                          pt2 = ps_t.tile([P, P], f32, tag="pt")
                          nc.tensor.transpose(pt2[:], hnew[:], ident_s[:])
                          nc.scalar.activation(hT[:, sl], pt2[:], AF.Copy)
                      # quarter-q AllGather as soon as its windows are done
                      if l < L - 1 and "noag" not in flags and _lr == LREPS - 1:
                          wdone = (g + 1) * GRPW
                          for q in range(NBANKS):
                              if wdone - GRPW < (q + 1) * WQ <= wdone:
                                  nc.gpsimd.collective_compute(
                                      "AllGather", OP.bypass,
                                      ins=[h_shard[l + 1][q * SQ:(q + 1) * SQ, :]],
                                      outs=[h_rep[l + 1][q * bank_rows:(q + 1) * bank_rows, :]],
                                      replica_groups=[list(range(NCORES))])

            # ---- pooling (hT holds final h; pad slots are zero) ----
            wsum = res.tile([P, W], f32)
            wmax = res.tile([P, W], f32)
            for w in range(W):
                sl = slice(w * P, (w + 1) * P)
                nc.vector.reduce_sum(wsum[:, w:w + 1], hT[:, sl],
                                     axis=mybir.AxisListType.X)
                nc.vector.reduce_max(wmax[:, w:w + 1], hT[:, sl],
                                     axis=mybir.AxisListType.X)
            gsum = sml.tile([P, GPC], f32, tag="gsum")
            gmax = sml.tile([P, GPC], f32, tag="gmax")
            for g in range(GPC):
                nc.vector.reduce_sum(gsum[:, g:g + 1], wsum[:, g * WG:(g + 1) * WG],
                                     axis=mybir.AxisListType.X)
                nc.vector.reduce_max(gmax[:, g:g + 1], wmax[:, g * WG:(g + 1) * WG],
                                     axis=mybir.AxisListType.X)
            icb = sml.tile([P, GPC], f32, tag="icb")
            nc.sync.dma_start(icb[:], bcast_row(invcnt_d[0, :], P, GPC))
            nc.vector.tensor_tensor(gsum[:], gsum[:], icb[:], OP.mult)

            # ---- trackster encoder (feature-major, GPC graphs) ----
            tsT_s = sml.tile([3, GPC], f32, tag="tsT")
            tsW1_s = sml.tile([3, HT], f32, tag="tsW1")
            tsW2_s = sml.tile([HT, HT], f32, tag="tsW2")
            tsb1_c = sml.tile([HT, 1], f32, tag="tsb1")
            tsb2_c = sml.tile([HT, 1], f32, tag="tsb2")
            nc.sync.dma_start(tsT_s[:], tsT_d[:])
            nc.sync.dma_start(tsW1_s[:], tsW1_d[:])
            nc.sync.dma_start(tsW2_s[:], tsW2_d[:])
            nc.sync.dma_start(tsb1_c[:], tsb1_d[:].rearrange("h -> h ()"))
            nc.sync.dma_start(tsb2_c[:], tsb2_d[:].rearrange("h -> h ()"))
            p1 = ps_hn.tile([HT, GPC], f32, tag="ph")
            nc.tensor.matmul(p1[:], lhsT=tsW1_s[:], rhs=tsT_s[:], start=True, stop=True)
            t1T = sml.tile([HT, GPC], f32, tag="t1T")
            nc.scalar.activation(t1T[:], p1[:], AF.Identity, bias=tsb1_c[:])
            # LN over HT in graph-major
            pg = ps_t.tile([GPC, HT], f32, tag="pt")
            nc.tensor.transpose(pg[:], t1T[:], ident_s[:HT, :HT])
            t1g = sml.tile([GPC, HT], f32, tag="t1g")
            nc.vector.tensor_copy(t1g[:], pg[:])
            tst = sml.tile([GPC, 6], f32, tag="tst6")
            tmv = sml.tile([GPC, 2], f32, tag="tsmv")
            nc.vector.bn_stats(tst[:], t1g[:])
            nc.vector.bn_aggr(tmv[:], tst[:])
            trs = sml.tile([GPC, 1], f32, tag="tsrstd")
            nc.scalar.activation(trs[:], tmv[:, 1:2], AF.Sqrt, bias=eps_c[:GPC, :])
            nc.vector.reciprocal(trs[:], trs[:])
            tlgb = sml.tile([GPC, HT], f32, tag="tlgb")
            tlbb = sml.tile([GPC, HT], f32, tag="tlbb")
            nc.sync.dma_start(tlgb[:], bcast_row(tslng_d[:], GPC, HT))
            nc.sync.dma_start(tlbb[:], bcast_row(tslnb_d[:], GPC, HT))
            nc.vector.scalar_tensor_tensor(t1g[:], t1g[:], tmv[:, 0:1], tlgb[:],
                                           OP.subtract, OP.mult)
            nc.vector.scalar_tensor_tensor(t1g[:], t1g[:], trs[:], tlbb[:],
                                           OP.mult, OP.add)
            nc.scalar.activation(t1g[:], t1g[:], AF.Relu)
            pr = ps_t.tile([HT, GPC], f32, tag="pt")
            nc.tensor.transpose(pr[:], t1g[:], ident_s[:GPC, :GPC])
            t1nT = sml.tile([HT, GPC], f32, tag="t1nT")
            nc.vector.tensor_copy(t1nT[:], pr[:])
            p2 = ps_hn.tile([HT, GPC], f32, tag="ph")
            nc.tensor.matmul(p2[:], lhsT=tsW2_s[:], rhs=t1nT[:], start=True, stop=True)
            t2T = sml.tile([HT, GPC], f32, tag="t2T")
            nc.scalar.activation(t2T[:], p2[:], AF.Identity, bias=tsb2_c[:])

            # ---- classifier ----
            PD = 2 * H + HT
            feat = sml.tile([GPC, PD], f32, tag="feat")
            pf = ps_t.tile([GPC, P], f32, tag="pt")
            nc.tensor.transpose(pf[:], gsum[:], ident_s[:])
            nc.vector.tensor_copy(feat[:, 0:H], pf[:])
            pf2 = ps_t.tile([GPC, P], f32, tag="pt")
            nc.tensor.transpose(pf2[:], gmax[:], ident_s[:])
            nc.vector.tensor_copy(feat[:, H:2 * H], pf2[:])
            pf3 = ps_t.tile([GPC, HT], f32, tag="pt")
            nc.tensor.transpose(pf3[:], t2T[:], ident_s[:HT, :HT])
            nc.vector.tensor_copy(feat[:, 2 * H:PD], pf3[:])
            # LN(PD)
            cst = sml.tile([GPC, 6], f32, tag="cst")
            cmv = sml.tile([GPC, 2], f32, tag="cmv")
            nc.vector.bn_stats(cst[:], feat[:])
            nc.vector.bn_aggr(cmv[:], cst[:])
            crs = sml.tile([GPC, 1], f32, tag="crs")
            nc.scalar.activation(crs[:], cmv[:, 1:2], AF.Sqrt, bias=eps_c[:GPC, :])
            nc.vector.reciprocal(crs[:], crs[:])
            cgb = sml.tile([GPC, PD], f32, tag="cgb")
            cbb = sml.tile([GPC, PD], f32, tag="cbb")
            nc.sync.dma_start(cgb[:], bcast_row(clng_d[:], GPC, PD))
            nc.sync.dma_start(cbb[:], bcast_row(clnb_d[:], GPC, PD))
            nc.vector.scalar_tensor_tensor(feat[:], feat[:], cmv[:, 0:1], cgb[:],
                                           OP.subtract, OP.mult)
            nc.vector.scalar_tensor_tensor(feat[:], feat[:], crs[:], cbb[:],
                                           OP.mult, OP.add)
            # z = relu(feat @ W1 + b1) in feature-major: zT [H, GPC]
            cb1_c = sml.tile([H, 1], f32, tag="cb1")
            nc.sync.dma_start(cb1_c[:], cb1_d[:].rearrange("h -> h ()"))
            pz = ps_hn.tile([H, GPC], f32, tag="ph")
            for j, (a, b_) in enumerate([(0, H), (H, 2 * H), (2 * H, PD)]):
                cW1j = sml.tile([b_ - a, H], f32, tag="cW1j", name=f"cW1j{j}")
                nc.sync.dma_start(cW1j[:], cW1_d[a:b_, :])
                pfj = ps_t.tile([b_ - a, GPC], f32, tag="pt")
                nc.tensor.transpose(pfj[:], feat[:, a:b_],
                                    ident_s[:GPC, :GPC])
                fTj = sml.tile([b_ - a, GPC], f32, tag="fTj")
                nc.vector.tensor_copy(fTj[:], pfj[:])
                nc.tensor.matmul(pz[:], lhsT=cW1j[:], rhs=fTj[:],
                                 start=(j == 0), stop=(j == 2))
            zT = sml.tile([H, GPC], f32, tag="zT")
            nc.scalar.activation(zT[:], pz[:], AF.Relu, bias=cb1_c[:])
            cW2_s = sml.tile([H, NCLS], f32, tag="cW2")
            nc.sync.dma_start(cW2_s[:], cW2_d[:])
            po = ps_hn.tile([GPC, NCLS], f32, tag="ph")
            nc.tensor.matmul(po[:], lhsT=zT[:], rhs=cW2_s[:], start=True, stop=True)
            ob = sml.tile([GPC, NCLS], f32, tag="ob")
            nc.sync.dma_start(ob[:], bcast_row(cb2_d[:], GPC, NCLS))
            outs = sml.tile([GPC, NCLS], f32, tag="outs")
            nc.vector.tensor_tensor(outs[:], po[:], ob[:], OP.add)
            nc.sync.dma_start(out_d[:], outs[:])

        for _rep in range(REPS):
            _pipeline()

    nc.compile()
    return nc


# ----------------------------------------------------------------------------
# entry point
# ----------------------------------------------------------------------------

def kernel(**inputs):
    from concourse.bass_utils import run_bass_kernel_spmd

    x = np.asarray(inputs["x"], np.float32)
    edge_index = np.asarray(inputs["edge_index"])
    batch = np.asarray(inputs["batch"])
    ts = np.asarray(inputs["ts"], np.float32)

    weights = {
        "enc_W": np.asarray(inputs["enc_W"], np.float32),
        "enc_b": np.asarray(inputs["enc_b"], np.float32),
        "sage_Wl": np.asarray(inputs["sage_Wl"], np.float16).reshape(L * H, H),
        "sage_bl": np.asarray(inputs["sage_bl"], np.float32),
        "sage_Wr": np.asarray(inputs["sage_Wr"], np.float32).reshape(L * H, H),
        "ln_g": np.asarray(inputs["ln_g"], np.float32),
        "ln_b": np.asarray(inputs["ln_b"], np.float32),
        "ts_W1": np.asarray(inputs["ts_W1"], np.float32),
        "ts_b1": np.asarray(inputs["ts_b1"], np.float32),
        "ts_lng": np.asarray(inputs["ts_lng"], np.float32),
        "ts_lnb": np.asarray(inputs["ts_lnb"], np.float32),
        "ts_W2": np.asarray(inputs["ts_W2"], np.float32),
        "ts_b2": np.asarray(inputs["ts_b2"], np.float32),
        "cls_lng": np.asarray(inputs["cls_lng"], np.float32),
        "cls_lnb": np.asarray(inputs["cls_lnb"], np.float32),
        "cls_W1": np.asarray(inputs["cls_W1"], np.float32),
        "cls_b1": np.asarray(inputs["cls_b1"], np.float32),
        "cls_W2": np.asarray(inputs["cls_W2"], np.float32),
        "cls_b2": np.asarray(inputs["cls_b2"], np.float32),
    }

    sched = _build_schedule(x, edge_index, batch)
    per_core = _host_inputs(sched, x, ts, weights)
    nc = _build_nc(sched)
    res = run_bass_kernel_spmd(nc, per_core, list(range(NCORES)), **_run_kwargs)
    if _res_hook is not None:
        _res_hook(res)
    return np.concatenate([res.results[c]["out"] for c in range(NCORES)], axis=0)


_run_kwargs = {}
_res_hook = None


# revision 28
# speedup vs baseline: 1.5830x; 1.0218x over previous
"""EnhancedGraphSAGE on 8 trn2 NeuronCores (Bass/Tile).

Sharding: 8 graphs per core (batch is sorted -> nodes graph-contiguous).
Each graph padded to G_slot slots (multiple of 128) with phantom nodes that
clone the graph's first node's features (no in-edges), so windows are
graph-pure with fully static shapes. h >= 0 always (relu encoder + relu
residual), so zeroing pad columns before pooling makes sum/max exact.
h is replicated across cores (fp16) via AllGather after the encoder and
after each SAGE layer.

Mean aggregation: per-core edges are grouped by (dst group of 4 windows,
src bank) into 128-edge chunks. dma_gather (int16 idx, 4 DRAM banks of the
fp16 replicated h, one SWDGE queue per bank) pulls h[src] rows into SBUF;
the PE accumulates aggT[f, slot-in-group] per 512-slot group as
gathered.T @ onehot where onehot[e, n] = (dlocal[e]==n) * invdeg[dst_e]
(fp16) is built on DVE with one fused tensor_scalar per chunk.
hn = agg@Wl + bl + h@Wr runs from aggT / resident hT (feature-major),
LN + relu + residual in node-major.
"""

import math
import os
from contextlib import ExitStack

import numpy as np

H = 128
HT = 64
NCLS = 8
L = 3
P = 128
NCORES = 8
GPC = 8  # graphs per core
GRPW = 4  # windows per psum group (512 dst slots)
GW = GRPW * P
SGG = 2  # groups per supergroup (one gather call per bank per supergroup)
MAX_BANK_ROWS = 32767
NBANKS = 4


# ----------------------------------------------------------------------------
# host-side schedule construction
# ----------------------------------------------------------------------------

def _build_schedule(x, edge_index, batch):
    N = x.shape[0]
    E = edge_index.shape[1]
    B = GPC * NCORES
    cnt = np.bincount(batch, minlength=B)
    assert cnt.min() > 0, "empty graph unsupported"
    gstart = np.zeros(B + 1, np.int64)
    np.cumsum(cnt, out=gstart[1:])
    G_slot = int(math.ceil(cnt.max() / P) * P)
    S = GPC * G_slot          # padded slots per core
    W = S // P                # windows per core
    WG = G_slot // P          # windows per graph
    assert S % GW == 0
    ngroups = S // GW
    bank_rows = int(math.ceil(NCORES * S / NBANKS))
    assert bank_rows <= MAX_BANK_ROWS

    g_of = batch.astype(np.int64)
    core_of_g = np.arange(B) // GPC
    slot_in_core_base = (np.arange(B) % GPC) * G_slot
    # per-core slot of real node n
    slot = slot_in_core_base[g_of] + (np.arange(N) - gstart[g_of])
    core_of_n = core_of_g[g_of]

    src = edge_index[0].astype(np.int64)
    dst = edge_index[1].astype(np.int64)
    deg = np.bincount(dst, minlength=N).astype(np.float64)
    invdeg_node = 1.0 / np.maximum(deg, 1.0)

    # bank = quarter of the per-core slot space; quarter q of every core's
    # shard is AllGathered into h_rep rows [q*bank_rows, (q+1)*bank_rows)
    # as [core, slot-in-quarter], so bank b only depends on collective b.
    SQ = S // NBANKS
    assert bank_rows == NCORES * SQ

    e_core = core_of_g[g_of[dst]]
    e_slot = slot[dst]
    e_inv = invdeg_node[dst]

    e_g = e_slot // GW        # dst group within core
    e_dl = e_slot % GW        # dst slot within group
    e_bank = slot[src] // SQ
    e_idx = core_of_n[src] * SQ + (slot[src] % SQ)

    # per (core, group, bank) cell edge lists; chunk count = max over cores
    key = ((e_core * ngroups + e_g) * NBANKS + e_bank).astype(np.int64)
    korder = np.argsort(key, kind="stable")
    ks = key[korder]
    bounds = np.searchsorted(ks, np.arange(NCORES * ngroups * NBANKS + 1))

    def cell_list(c, g, b):
        k = (c * ngroups + g) * NBANKS + b
        return korder[bounds[k]:bounds[k + 1]]

    cell_chunks = np.zeros((ngroups, NBANKS), np.int64)
    for g in range(ngroups):
        for b in range(NBANKS):
            m = max(len(cell_list(c, g, b)) for c in range(NCORES))
            cell_chunks[g, b] = (m + P - 1) // P
    nchunks = int(cell_chunks.sum())

    # supergroups: SGG consecutive groups share one gather call per bank
    sgs = [list(range(s, min(s + SGG, ngroups))) for s in range(0, ngroups, SGG)]
    # chunk base of cell (g, b) within its (sg, b) call
    cell_base = {}
    call_nch = {}  # (si, b) -> chunks in call
    for si, sg in enumerate(sgs):
        for b in range(NBANKS):
            ofs = 0
            for g in sg:
                cell_base[(g, b)] = ofs
                ofs += int(cell_chunks[g, b])
            call_nch[(si, b)] = ofs

    # pack per-core idx / dlocal / invdegE in emission order
    # (each call's idx region 64B-aligned: 32 int16 cols)
    def _acols(nch):
        return -(-int(nch) * P // 16 // 32) * 32

    total_idx_cols = sum(_acols(call_nch[(si, b)])
                         for si in range(len(sgs)) for b in range(NBANKS))
    idx16 = np.zeros((NCORES, 128, total_idx_cols), np.int16)
    dlocal = np.full((NCORES, P, nchunks), 30000.0, np.float32)
    invdegE = np.zeros((NCORES, P, nchunks), np.float32)

    # global chunk offset of cell (g, b) in dlocal/invdegE, in emission order
    chunk_ofs = {}
    ofs = 0
    for si, sg in enumerate(sgs):
        for b in range(NBANKS):
            for g in sg:
                chunk_ofs[(g, b)] = ofs
                ofs += int(cell_chunks[g, b])
    assert ofs == nchunks

    for c in range(NCORES):
        colofs = 0
        for si, sg in enumerate(sgs):
            for b in range(NBANKS):
                nch_call = call_nch[(si, b)]
                if nch_call == 0:
                    continue
                vals = np.zeros(nch_call * P, np.int64)
                for g in sg:
                    nch = int(cell_chunks[g, b])
                    if nch == 0:
                        continue
                    lst = cell_list(c, g, b)
                    n = len(lst)
                    cb = cell_base[(g, b)]
                    vals[cb * P: cb * P + n] = e_idx[lst]
                    dlf = np.full(nch * P, 999.0, np.float32)
                    ivf = np.zeros(nch * P, np.float32)
                    dlf[:n] = e_dl[lst]
                    ivf[:n] = e_inv[lst]
                    co = chunk_ofs[(g, b)]
                    dlocal[c, :, co:co + nch] = dlf.reshape(nch, P).T
                    invdegE[c, :, co:co + nch] = ivf.reshape(nch, P).T
                ncols = nch_call * P // 16
                wrapped = vals.reshape(ncols, 16).T.astype(np.int16)
                for r in range(8):
                    idx16[c, r * 16:(r + 1) * 16, colofs:colofs + ncols] = wrapped
                colofs += _acols(nch_call)

    # first/last (bank, chunk) per group for PSUM start/stop
    grp_first = {}
    grp_last = {}
    for g in range(ngroups):
        nz = [(b, int(cell_chunks[g, b])) for b in range(NBANKS)
              if cell_chunks[g, b] > 0]
        if nz:
            grp_first[g] = (nz[0][0], 0)
            grp_last[g] = (nz[-1][0], nz[-1][1] - 1)

    return dict(
        N=N, E=E, B=B, cnt=cnt, gstart=gstart, G_slot=G_slot, S=S, W=W,
        WG=WG, bank_rows=bank_rows, slot=slot, SQ=SQ,
        cell_chunks=cell_chunks, nchunks=nchunks, sgs=sgs,
        cell_base=cell_base, call_nch=call_nch, chunk_ofs=chunk_ofs,
        idx16=idx16, dlocal=dlocal, invdegE=invdegE,
        total_idx_cols=total_idx_cols,
        grp_first=grp_first, grp_last=grp_last, ngroups=ngroups,
    )


def _host_inputs(sched, x, ts, weights):
    """Per-core input dicts (plus shared tensors replicated)."""
    S, G_slot = sched["S"], sched["G_slot"]
    cnt, gstart = sched["cnt"], sched["gstart"]
    slot = sched["slot"]

    xT = np.zeros((NCORES, 4, S), np.float32)
    g_all = np.repeat(np.arange(sched["B"]), cnt)
    for c in range(NCORES):
        sel = (g_all // GPC) == c
        xT[c, :, slot[sel]] = x[sel]
    # phantoms copy n0's features (keeps pad h finite; no in-edges)
    for g in range(sched["B"]):
        c = g // GPC
        base = (g % GPC) * G_slot
        nph = G_slot - cnt[g]
        if nph > 0:
            xT[c, :, base + cnt[g]: base + G_slot] = x[gstart[g]][:, None]

    invcnt = np.zeros((NCORES, GPC), np.float32)
    for g in range(sched["B"]):
        invcnt[g // GPC, g % GPC] = 1.0 / cnt[g]

    W = sched["W"]
    padmask = np.zeros((NCORES, P, W), np.float32)
    for g in range(sched["B"]):
        c = g // GPC
        base = (g % GPC) * G_slot
        real = np.arange(base, base + cnt[g])
        padmask[c, real % P, real // P] = 1.0

    iota512 = np.tile(np.arange(GW, dtype=np.float16), (P, 1))
    ident = np.eye(P, dtype=np.float32)
    ident16 = np.eye(P, dtype=np.float16)

    per_core = []
    for c in range(NCORES):
        d = {
            "xT": np.ascontiguousarray(xT[c]),
            "gidx": np.ascontiguousarray(sched["idx16"][c]),
            "dlocal": np.ascontiguousarray(sched["dlocal"][c]),
            "invdegE": np.ascontiguousarray(sched["invdegE"][c]),
            "tsT": np.ascontiguousarray(ts[c * GPC:(c + 1) * GPC].T.astype(np.float32)),
            "invcnt": invcnt[c:c + 1],
            "padmask": np.ascontiguousarray(padmask[c]),
            "iota512": iota512,
            "ident": ident,
            "ident16": ident16,
        }
        for k, v in weights.items():
            d[k] = v
        per_core.append(d)
    return per_core


# ----------------------------------------------------------------------------
# bass program
# ----------------------------------------------------------------------------

def _build_nc(sched):
    import concourse.bacc as bacc
    import concourse.bass as bass
    import concourse.mybir as mybir
    import concourse.tile as tile
    from concourse import library_config

    f32 = mybir.dt.float32
    f16 = mybir.dt.float16
    AF = mybir.ActivationFunctionType
    OP = mybir.AluOpType

    S, W, WG = sched["S"], sched["W"], sched["WG"]
    SQ = sched["SQ"]
    WQ = W // NBANKS
    bank_rows = sched["bank_rows"]
    ngroups = sched["ngroups"]
    cell_chunks = sched["cell_chunks"]
    sgs = sched["sgs"]
    cell_base = sched["cell_base"]
    call_nch = sched["call_nch"]
    chunk_ofs = sched["chunk_ofs"]
    nchunks = sched["nchunks"]
    total_idx_cols = sched["total_idx_cols"]
    grp_first, grp_last = sched["grp_first"], sched["grp_last"]

    stage = os.environ.get("GNN_STAGE", "full")
    flags = set(stage.split("+"))
    nc = bacc.Bacc("TRN2", target_bir_lowering=False, num_swdge_queues=NBANKS)

    def din(name, shape, dtype=f32):
        return nc.dram_tensor(name, shape, dtype, kind="ExternalInput")

    xT_d = din("xT", [4, S])
    gidx_d = din("gidx", [128, total_idx_cols], mybir.dt.int16)
    dlocal_d = din("dlocal", [P, nchunks])
    invdegE_d = din("invdegE", [P, nchunks])
    tsT_d = din("tsT", [3, GPC])
    invcnt_d = din("invcnt", [1, GPC])
    padmask_d = din("padmask", [P, W])
    iota512_d = din("iota512", [P, GW], f16)
    ident_d = din("ident", [P, P])
    ident16_d = din("ident16", [P, P], f16)
    encW_d = din("enc_W", [4, H])
    encb_d = din("enc_b", [H])
    Wl_d = din("sage_Wl", [L * H, H], f16)
    bl_d = din("sage_bl", [L, H])
    Wr_d = din("sage_Wr", [L * H, H], f16)
    lng_d = din("ln_g", [L, H])
    lnb_d = din("ln_b", [L, H])
    tsW1_d = din("ts_W1", [3, HT])
    tsb1_d = din("ts_b1", [HT])
    tslng_d = din("ts_lng", [HT])
    tslnb_d = din("ts_lnb", [HT])
    tsW2_d = din("ts_W2", [HT, HT])
    tsb2_d = din("ts_b2", [HT])
    clng_d = din("cls_lng", [2 * H + HT])
    clnb_d = din("cls_lnb", [2 * H + HT])
    cW1_d = din("cls_W1", [2 * H + HT, H])
    cb1_d = din("cls_b1", [H])
    cW2_d = din("cls_W2", [H, NCLS])
    cb2_d = din("cls_b2", [NCLS])
    out_d = nc.dram_tensor("out", [GPC, NCLS], f32, kind="ExternalOutput")

    h_shard = [nc.dram_tensor(f"h_shard{l}", [S, H], f16) for l in range(L)]
    h_rep = [nc.dram_tensor(f"h_rep{l}", [NCORES * S, H], f16,
                            addr_space="Shared") for l in range(L)]
    # one-hot cache: built on DVE in layer 0, streamed back in layers 1+
    # layout [partition, chunk*GW] so a run of chunks is one 2D DMA
    oh_dram = nc.dram_tensor("oh_cache", [P, nchunks * GW], f16)

    def bcast_row(dram_ap, npart, width):
        # AP reading a [width] or [1,width] dram row replicated across npart partitions
        return bass.AP(tensor=dram_ap.tensor, offset=dram_ap.offset,
                       ap=[[0, npart]] + dram_ap.ap[-1:])

    with tile.TileContext(nc) as tc, ExitStack() as ctx:
        res = ctx.enter_context(tc.tile_pool(name="res", bufs=1))
        gath = ctx.enter_context(tc.tile_pool(name="gath", bufs=12))
        oh = ctx.enter_context(tc.tile_pool(name="oh", bufs=4))
        stg = ctx.enter_context(tc.tile_pool(name="stg", bufs=4))
        sml = ctx.enter_context(tc.tile_pool(name="sml", bufs=2))
        ps_agg = ctx.enter_context(tc.tile_pool(name="ps_agg", bufs=2, space="PSUM"))
        ps_hn = ctx.enter_context(tc.tile_pool(name="ps_hn", bufs=2, space="PSUM"))
        ps_t = ctx.enter_context(tc.tile_pool(name="ps_t", bufs=2, space="PSUM"))

        nc.gpsimd.load_library(library_config.mlp)

        # ---- residents ----
        hT = res.tile([P, S], f16)                      # feature-major h shard
        gidx_s = res.tile([128, total_idx_cols], mybir.dt.int16)
        dl_s = res.tile([P, nchunks], f32)
        iv_s = res.tile([P, nchunks], f32)
        iota_s = res.tile([P, GW], f16)
        ident_s = res.tile([P, P], f32)
        ident16_s = res.tile([P, P], f16)
        encW_s = res.tile([4, H], f32)
        encb_c = res.tile([P, 1], f32)
        eps_c = res.tile([P, 1], f32)
        padmask_s = res.tile([P, W], f32)
        nc.sync.dma_start(padmask_s[:], padmask_d[:])
        nc.sync.dma_start(gidx_s[:], gidx_d[:])
        nc.sync.dma_start(dl_s[:], dlocal_d[:])
        nc.sync.dma_start(iv_s[:], invdegE_d[:])
        nc.sync.dma_start(iota_s[:], iota512_d[:])
        nc.sync.dma_start(ident_s[:], ident_d[:])
        nc.sync.dma_start(ident16_s[:], ident16_d[:])
        nc.sync.dma_start(encW_s[:], encW_d[:])
        nc.sync.dma_start(encb_c[:], encb_d.ap().rearrange("h -> h ()"))
        nc.vector.memset(eps_c[:], 1e-5)

        REPS = int(os.environ.get("GNN_REPS", "1"))

        def _pipeline():
            # ---- encoder: hT = relu(enc_W.T @ xT + b) ----
            for w in range(W):
                sl = slice(w * P, (w + 1) * P)
                xw = stg.tile([4, P], f32, tag="xw")
                nc.sync.dma_start(xw[:], xT_d[:, sl])
                ps = ps_hn.tile([P, P], f32, tag="ph")
                nc.tensor.matmul(ps[:], lhsT=encW_s[:], rhs=xw[:],
                                 start=True, stop=True)
                nc.scalar.activation(hT[:, sl], ps[:], AF.Relu, bias=encb_c[:])
                pt = ps_t.tile([P, P], f16, tag="pt16")
                nc.tensor.transpose(pt[:], hT[:, sl], ident16_s[:])
                st = stg.tile([P, P], f16, tag="st16")
                nc.scalar.activation(st[:], pt[:], AF.Copy)
                nc.sync.dma_start(h_shard[0][sl, :], st[:])
                if (w + 1) % WQ == 0 and not flags & {"noag", "nolayers"}:
                    q = w // WQ
                    nc.gpsimd.collective_compute(
                        "AllGather", OP.bypass,
                        ins=[h_shard[0][q * SQ:(q + 1) * SQ, :]],
                        outs=[h_rep[0][q * bank_rows:(q + 1) * bank_rows, :]],
                        replica_groups=[list(range(NCORES))])

            # ---- SAGE layers ----
            for l in range(L if "nolayers" not in flags else 0):
                Wl_s = sml.tile([H, H], f16, tag="wl")
                Wr_s = sml.tile([H, H], f16, tag="wr")
                blb = sml.tile([P, H], f32, tag="blb")
                gb = sml.tile([P, H], f32, tag="gb")
                bb = sml.tile([P, H], f32, tag="bb")
                nc.sync.dma_start(Wl_s[:], Wl_d[l * H:(l + 1) * H, :])
                nc.sync.dma_start(Wr_s[:], Wr_d[l * H:(l + 1) * H, :])
                nc.sync.dma_start(blb[:], bcast_row(bl_d[l, :], P, H))
                nc.sync.dma_start(gb[:], bcast_row(lng_d[l, :], P, H))
                nc.sync.dma_start(bb[:], bcast_row(lnb_d[l, :], P, H))

                LREPS = int(os.environ.get("GNN_LREPS", "1"))
                for _lr in range(LREPS):
                  colofs = 0
                  for si, sg in enumerate(sgs):
                    gtiles = {}
                    for b in range(NBANKS):
                        nch_call = call_nch[(si, b)]
                        if nch_call == 0:
                            continue
                        ncols = nch_call * P // 16
                        acols = -(-ncols // 32) * 32
                        if "nogather" not in flags:
                            gt = gath.tile([P, nch_call, P], f16, tag="gath")
                            nc.gpsimd.dma_gather(
                                gt[:], h_rep[l][b * bank_rows:(b + 1) * bank_rows, :],
                                gidx_s[:, colofs:colofs + ncols],
                                nch_call * P, nch_call * P, H,
                                single_packet=(nch_call * P <= 1024),
                                queue_num=b)
                            gtiles[b] = gt
                        colofs += acols
                    for g in sg:
                      # chunk matmuls: bank-major, accumulating aggT per group
                      psw = None
                      if g in grp_first and not flags & {"nogather", "gatheronly"}:
                          psw = ps_agg.tile([P, GW], f32, tag="aggw")
                      for b in range(NBANKS):
                          if b not in gtiles or "gatheronly" in flags:
                              continue
                          assert "nogather" not in flags
                          nch = int(cell_chunks[g, b])
                          cb = cell_base[(g, b)]
                          co = chunk_ofs[(g, b)]
                          ohcell = oh.tile([P, nch, GW], f16, tag="ohc")
                          if l == 0 and _lr == 0:
                              for c in range(nch):
                                  nc.vector.tensor_scalar(
                                      ohcell[:, c, :], iota_s[:],
                                      dl_s[:, co + c:co + c + 1],
                                      iv_s[:, co + c:co + c + 1],
                                      OP.is_equal, OP.mult)
                              nc.sync.dma_start(
                                  oh_dram[:, co * GW:(co + nch) * GW],
                                  ohcell[:])
                          else:
                              nc.sync.dma_start(
                                  ohcell[:],
                                  oh_dram[:, co * GW:(co + nch) * GW])
                          for c in range(nch):
                              nc.tensor.matmul(
                                  psw[:], lhsT=gtiles[b][:, cb + c, :],
                                  rhs=ohcell[:, c, :],
                                  start=(grp_first[g] == (b, c)),
                                  stop=(grp_last[g] == (b, c)))
                      # group tail: aggT -> fp16, then per-window hn
                      aggT = stg.tile([P, GW], f16, tag="aggT")
                      if psw is not None:
                          nc.scalar.activation(aggT[:], psw[:], AF.Copy)
                      else:
                          nc.vector.memset(aggT[:], 0.0)
                      for wi in range(GRPW):
                          w = g * GRPW + wi
                          sl = slice(w * P, (w + 1) * P)
                          ph = ps_hn.tile([P, H], f32, tag="ph")
                          nc.tensor.matmul(ph[:], lhsT=aggT[:, wi * P:(wi + 1) * P],
                                           rhs=Wl_s[:], start=True, stop=False)
                          nc.tensor.matmul(ph[:], lhsT=hT[:, sl], rhs=Wr_s[:],
                                           start=False, stop=True)
                          hn = stg.tile([P, H], f32, tag="hn_s")
                          nc.vector.tensor_tensor(hn[:], ph[:], blb[:], OP.add)
                          stats = sml.tile([P, 6], f32, tag="st6")
                          mv = sml.tile([P, 2], f32, tag="mv")
                          nc.vector.bn_stats(stats[:], hn[:])
                          nc.vector.bn_aggr(mv[:], stats[:])
                          rstd = sml.tile([P, 1], f32, tag="rstd")
                          nc.scalar.activation(rstd[:], mv[:, 1:2], AF.Sqrt,
                                               bias=eps_c[:])
                          nc.vector.reciprocal(rstd[:], rstd[:])
                          t1 = stg.tile([P, H], f32, tag="t1")
                          nc.vector.scalar_tensor_tensor(
                              t1[:], hn[:], mv[:, 0:1], gb[:],
                              OP.subtract, OP.mult)
                          nc.vector.scalar_tensor_tensor(
                              t1[:], t1[:], rstd[:], bb[:], OP.mult, OP.add)
                          nc.scalar.activation(t1[:], t1[:], AF.Relu)
                          pt = ps_t.tile([P, P], f16, tag="pt16")
                          nc.tensor.transpose(pt[:], hT[:, sl], ident16_s[:])
                          hnew = stg.tile([P, H], f32, tag="hnew")
                          nc.vector.tensor_tensor(hnew[:], t1[:], pt[:], OP.add)
                          if l < L - 1:
                              h16 = stg.tile([P, H], f16, tag="h16")
                              nc.scalar.activation(h16[:], hnew[:], AF.Copy)
                              nc.sync.dma_start(h_shard[l + 1][sl, :], h16[:])
                          else:
                              # zero pad slots (h >= 0, so pooling sum/max
                              # over the padded window stays exact)
                              hm = stg.tile([P, H], f32, tag="hmask")
                              nc.scalar.activation(hm[:], hnew[:], AF.Copy,
                                                   scale=padmask_s[:, w:w + 1])
                              hnew = hm
                          pt2 = ps_t.tile([P, P], f32, tag="pt")
                          nc.tensor.transpose(pt2[:], hnew[:], ident_s[:])
                          nc.scalar.activation(hT[:, sl], pt2[:], AF.Copy)
                      # quarter-q AllGather as soon as its windows are done
                      if l < L - 1 and "noag" not in flags and _lr == LREPS - 1:
                          wdone = (g + 1) * GRPW
                          for q in range(NBANKS):
                              if wdone - GRPW < (q + 1) * WQ <= wdone:
                                  nc.gpsimd.collective_compute(
                                      "AllGather", OP.bypass,
                                      ins=[h_shard[l + 1][q * SQ:(q + 1) * SQ, :]],
                                      outs=[h_rep[l + 1][q * bank_rows:(q + 1) * bank_rows, :]],
                                      replica_groups=[list(range(NCORES))])

            # ---- pooling (hT holds final h; pad slots are zero) ----
            wsum = res.tile([P, W], f32)
            wmax = res.tile([P, W], f32)
            for w in range(W):
                sl = slice(w * P, (w + 1) * P)
                nc.vector.reduce_sum(wsum[:, w:w + 1], hT[:, sl],
                                     axis=mybir.AxisListType.X)
                nc.vector.reduce_max(wmax[:, w:w + 1], hT[:, sl],
                                     axis=mybir.AxisListType.X)
            gsum = sml.tile([P, GPC], f32, tag="gsum")
            gmax = sml.tile([P, GPC], f32, tag="gmax")
            for g in range(GPC):
                nc.vector.reduce_sum(gsum[:, g:g + 1], wsum[:, g * WG:(g + 1) * WG],
                                     axis=mybir.AxisListType.X)
                nc.vector.reduce_max(gmax[:, g:g + 1], wmax[:, g * WG:(g + 1) * WG],
                                     axis=mybir.AxisListType.X)
            icb = sml.tile([P, GPC], f32, tag="icb")
            nc.sync.dma_start(icb[:], bcast_row(invcnt_d[0, :], P, GPC))
            nc.vector.tensor_tensor(gsum[:], gsum[:], icb[:], OP.mult)

            # ---- trackster encoder (feature-major, GPC graphs) ----
            tsT_s = sml.tile([3, GPC], f32, tag="tsT")
            tsW1_s = sml.tile([3, HT], f32, tag="tsW1")
            tsW2_s = sml.tile([HT, HT], f32, tag="tsW2")
            tsb1_c = sml.tile([HT, 1], f32, tag="tsb1")
            tsb2_c = sml.tile([HT, 1], f32, tag="tsb2")
            nc.sync.dma_start(tsT_s[:], tsT_d[:])
            nc.sync.dma_start(tsW1_s[:], tsW1_d[:])
            nc.sync.dma_start(tsW2_s[:], tsW2_d[:])
            nc.sync.dma_start(tsb1_c[:], tsb1_d[:].rearrange("h -> h ()"))
            nc.sync.dma_start(tsb2_c[:], tsb2_d[:].rearrange("h -> h ()"))
            p1 = ps_hn.tile([HT, GPC], f32, tag="ph")
            nc.tensor.matmul(p1[:], lhsT=tsW1_s[:], rhs=tsT_s[:], start=True, stop=True)
            t1T = sml.tile([HT, GPC], f32, tag="t1T")
            nc.scalar.activation(t1T[:], p1[:], AF.Identity, bias=tsb1_c[:])
            # LN over HT in graph-major
            pg = ps_t.tile([GPC, HT], f32, tag="pt")
            nc.tensor.transpose(pg[:], t1T[:], ident_s[:HT, :HT])
            t1g = sml.tile([GPC, HT], f32, tag="t1g")
            nc.vector.tensor_copy(t1g[:], pg[:])
            tst = sml.tile([GPC, 6], f32, tag="tst6")
            tmv = sml.tile([GPC, 2], f32, tag="tsmv")
            nc.vector.bn_stats(tst[:], t1g[:])
            nc.vector.bn_aggr(tmv[:], tst[:])
            trs = sml.tile([GPC, 1], f32, tag="tsrstd")
            nc.scalar.activation(trs[:], tmv[:, 1:2], AF.Sqrt, bias=eps_c[:GPC, :])
            nc.vector.reciprocal(trs[:], trs[:])
            tlgb = sml.tile([GPC, HT], f32, tag="tlgb")
            tlbb = sml.tile([GPC, HT], f32, tag="tlbb")
            nc.sync.dma_start(tlgb[:], bcast_row(tslng_d[:], GPC, HT))
            nc.sync.dma_start(tlbb[:], bcast_row(tslnb_d[:], GPC, HT))
            nc.vector.scalar_tensor_tensor(t1g[:], t1g[:], tmv[:, 0:1], tlgb[:],
                                           OP.subtract, OP.mult)
            nc.vector.scalar_tensor_tensor(t1g[:], t1g[:], trs[:], tlbb[:],
                                           OP.mult, OP.add)
            nc.scalar.activation(t1g[:], t1g[:], AF.Relu)
            pr = ps_t.tile([HT, GPC], f32, tag="pt")
            nc.tensor.transpose(pr[:], t1g[:], ident_s[:GPC, :GPC])
            t1nT = sml.tile([HT, GPC], f32, tag="t1nT")
            nc.vector.tensor_copy(t1nT[:], pr[:])
            p2 = ps_hn.tile([HT, GPC], f32, tag="ph")
            nc.tensor.matmul(p2[:], lhsT=tsW2_s[:], rhs=t1nT[:], start=True, stop=True)
            t2T = sml.tile([HT, GPC], f32, tag="t2T")
            nc.scalar.activation(t2T[:], p2[:], AF.Identity, bias=tsb2_c[:])

            # ---- classifier ----
            PD = 2 * H + HT
            feat = sml.tile([GPC, PD], f32, tag="feat")
            pf = ps_t.tile([GPC, P], f32, tag="pt")
            nc.tensor.transpose(pf[:], gsum[:], ident_s[:])
            nc.vector.tensor_copy(feat[:, 0:H], pf[:])
            pf2 = ps_t.tile([GPC, P], f32, tag="pt")
            nc.tensor.transpose(pf2[:], gmax[:], ident_s[:])
            nc.vector.tensor_copy(feat[:, H:2 * H], pf2[:])
            pf3 = ps_t.tile([GPC, HT], f32, tag="pt")
            nc.tensor.transpose(pf3[:], t2T[:], ident_s[:HT, :HT])
            nc.vector.tensor_copy(feat[:, 2 * H:PD], pf3[:])
            # LN(PD)
            cst = sml.tile([GPC, 6], f32, tag="cst")
            cmv = sml.tile([GPC, 2], f32, tag="cmv")
            nc.vector.bn_stats(cst[:], feat[:])
            nc.vector.bn_aggr(cmv[:], cst[:])
            crs = sml.tile([GPC, 1], f32, tag="crs")
            nc.scalar.activation(crs[:], cmv[:, 1:2], AF.Sqrt, bias=eps_c[:GPC, :])
            nc.vector.reciprocal(crs[:], crs[:])
            cgb = sml.tile([GPC, PD], f32, tag="cgb")
            cbb = sml.tile([GPC, PD], f32, tag="cbb")
            nc.sync.dma_start(cgb[:], bcast_row(clng_d[:], GPC, PD))
            nc.sync.dma_start(cbb[:], bcast_row(clnb_d[:], GPC, PD))
            nc.vector.scalar_tensor_tensor(feat[:], feat[:], cmv[:, 0:1], cgb[:],
                                           OP.subtract, OP.mult)
            nc.vector.scalar_tensor_tensor(feat[:], feat[:], crs[:], cbb[:],
                                           OP.mult, OP.add)
            # z = relu(feat @ W1 + b1) in feature-major: zT [H, GPC]
            cb1_c = sml.tile([H, 1], f32, tag="cb1")
            nc.sync.dma_start(cb1_c[:], cb1_d[:].rearrange("h -> h ()"))
            pz = ps_hn.tile([H, GPC], f32, tag="ph")
            for j, (a, b_) in enumerate([(0, H), (H, 2 * H), (2 * H, PD)]):
                cW1j = sml.tile([b_ - a, H], f32, tag="cW1j", name=f"cW1j{j}")
                nc.sync.dma_start(cW1j[:], cW1_d[a:b_, :])
                pfj = ps_t.tile([b_ - a, GPC], f32, tag="pt")
                nc.tensor.transpose(pfj[:], feat[:, a:b_],
                                    ident_s[:GPC, :GPC])
                fTj = sml.tile([b_ - a, GPC], f32, tag="fTj")
                nc.vector.tensor_copy(fTj[:], pfj[:])
                nc.tensor.matmul(pz[:], lhsT=cW1j[:], rhs=fTj[:],
                                 start=(j == 0), stop=(j == 2))
            zT = sml.tile([H, GPC], f32, tag="zT")
            nc.scalar.activation(zT[:], pz[:], AF.Relu, bias=cb1_c[:])
            cW2_s = sml.tile([H, NCLS], f32, tag="cW2")
            nc.sync.dma_start(cW2_s[:], cW2_d[:])
            po = ps_hn.tile([GPC, NCLS], f32, tag="ph")
            nc.tensor.matmul(po[:], lhsT=zT[:], rhs=cW2_s[:], start=True, stop=True)
            ob = sml.tile([GPC, NCLS], f32, tag="ob")
            nc.sync.dma_start(ob[:], bcast_row(cb2_d[:], GPC, NCLS))
            outs = sml.tile([GPC, NCLS], f32, tag="outs")
            nc.vector.tensor_tensor(outs[:], po[:], ob[:], OP.add)
            nc.sync.dma_start(out_d[:], outs[:])

        for _rep in range(REPS):
            _pipeline()

    nc.compile()
    return nc


# ----------------------------------------------------------------------------
# entry point
# ----------------------------------------------------------------------------

def kernel(**inputs):
    from concourse.bass_utils import run_bass_kernel_spmd

    x = np.asarray(inputs["x"], np.float32)
    edge_index = np.asarray(inputs["edge_index"])
    batch = np.asarray(inputs["batch"])
    ts = np.asarray(inputs["ts"], np.float32)

    weights = {
        "enc_W": np.asarray(inputs["enc_W"], np.float32),
        "enc_b": np.asarray(inputs["enc_b"], np.float32),
        "sage_Wl": np.asarray(inputs["sage_Wl"], np.float16).reshape(L * H, H),
        "sage_bl": np.asarray(inputs["sage_bl"], np.float32),
        "sage_Wr": np.asarray(inputs["sage_Wr"], np.float16).reshape(L * H, H),
        "ln_g": np.asarray(inputs["ln_g"], np.float32),
        "ln_b": np.asarray(inputs["ln_b"], np.float32),
        "ts_W1": np.asarray(inputs["ts_W1"], np.float32),
        "ts_b1": np.asarray(inputs["ts_b1"], np.float32),
        "ts_lng": np.asarray(inputs["ts_lng"], np.float32),
        "ts_lnb": np.asarray(inputs["ts_lnb"], np.float32),
        "ts_W2": np.asarray(inputs["ts_W2"], np.float32),
        "ts_b2": np.asarray(inputs["ts_b2"], np.float32),
        "cls_lng": np.asarray(inputs["cls_lng"], np.float32),
        "cls_lnb": np.asarray(inputs["cls_lnb"], np.float32),
        "cls_W1": np.asarray(inputs["cls_W1"], np.float32),
        "cls_b1": np.asarray(inputs["cls_b1"], np.float32),
        "cls_W2": np.asarray(inputs["cls_W2"], np.float32),
        "cls_b2": np.asarray(inputs["cls_b2"], np.float32),
    }

    sched = _build_schedule(x, edge_index, batch)
    per_core = _host_inputs(sched, x, ts, weights)
    nc = _build_nc(sched)
    res = run_bass_kernel_spmd(nc, per_core, list(range(NCORES)), **_run_kwargs)
    if _res_hook is not None:
        _res_hook(res)
    return np.concatenate([res.results[c]["out"] for c in range(NCORES)], axis=0)


_run_kwargs = {}
_res_hook = None
